# revision 72
# baseline (speedup 1.0000x reference)
"""2-layer GATv2 (PyG GATv2Conv semantics) on 8 Trainium2 NeuronCores.

Strategy (v3):
  - Nodes sharded across 8 cores; per-core greedy 2D packing balances each
    destination tile's lo/hi in-edge counts (lo/hi = source node group, one
    AllGather chunk / int16-indexable table half each).
  - x is shipped pre-transposed; layer-1 projections are 6 accumulating
    matmuls per node tile with a combined [W1l|W1r] moving operand.
  - xl tables AllGathered HBM->HBM per group, overlapped with compute.
  - Per destination tile, incoming-edge source rows are fetched with
    dma_gather (int16 idx; two table halves; GB node tiles per call).
    SWDGE descriptor generation is serial on the Q7 at ~9ns/row and is the
    hard floor of this design -- hence self-loops are NOT gathered: each
    node tile has a dedicated slot-aligned self tile computed from resident
    xl/xr SBUF copies (DVE add + Prelu + identity-scatter), which also keeps
    pad-slot denominators positive (no NaNs).  Both one-hots (oh_ne dst
    -major, oh_en edge-major) are precomputed on host, streamed from HBM.
  - Per half (K edge tiles): K z-matmul pairs (one-hot xr broadcast +
    identity gx inject) into a shared PSUM chunk, ONE batched Prelu per <=3
    tiles, batched att mult / reduce / exp / msg mult over the half, then K
    scatter matmuls accumulate numerator+denominator in PSUM.
  - elu(x) = relu(x) - relu(1 - exp(x)): 3 ACT ops + 1 DVE op.
  - Softmax skips max-subtraction (scores O(1)); log_softmax likewise.
  - Layer 2 (heads=1, 16 ch) repeats the edge structure on a 256B-row
    table; epilogue division/log-softmax moved to ACT (scale/bias) where
    possible.

kernel(**inputs) takes FULL inputs, returns FULL outputs.
"""

import os
import sys

if "/opt/trn_rl_repo" not in sys.path:
    sys.path.insert(0, "/opt/trn_rl_repo")

import numpy as np
import ml_dtypes

NC = 8          # cores
P = 128         # partitions
NEG_SLOPE = 0.2
NGRP = 2        # AllGather chunks

_plan_cache = {}


# --------------------------------------------------------------------------
# host-side graph preprocessing
# --------------------------------------------------------------------------

def _snake(order, nbins):
    n = len(order)
    ids = np.arange(n)
    round_ = ids // nbins
    pos = ids % nbins
    b = np.where(round_ % 2 == 0, pos, nbins - 1 - pos)
    out = np.empty(n, np.int64)
    out[:] = b
    return out


def _preprocess(N, E, edge_index):
    # Two rank-groups (= AllGather chunks = table halves), 25 tile-ranks each.
    # (An unequal 40/60 split was tried to start the gather stream earlier; it
    # is 270us WORSE: the bigger second AllGather delays the hi-half tables
    # that phase B consumes tile-by-tile. Equal halves are optimal.)
    NTG = ((N + 2 * NC - 1) // (2 * NC) + P - 1) // P      # tiles per group
    NT = 2 * NTG
    NPC = NT * P
    NTG0 = NT // 2
    NTG1 = NT - NTG0
    NTGS = (NTG0, NTG1)
    NTGM = max(NTGS)
    TBL_G = NC * NTGM * P                                  # rows per half-table
    assert TBL_G < 32768, "table half must fit int16 row indices"

    # self-loops are handled by a dedicated slot-aligned "self tile" per node
    # tile in the kernel (no gather needed) -- only real edges go in lists.
    src = edge_index[0].astype(np.int64)
    dst = edge_index[1].astype(np.int64)
    deg = np.bincount(dst, minlength=N)

    # --- group assignment (fixes each edge's table half), then cores within
    # each group balanced by OWN-group in-degree (the heavy, self-loop half)
    order = np.argsort(-deg, kind="stable")
    grp_of = np.empty(N, np.int64)
    grp_of[order] = _snake(order, 2)
    e_own = grp_of[src] == grp_of[dst]
    deg_own = np.bincount(dst[e_own], minlength=N)
    deg_oth = deg - deg_own
    core_of = np.empty(N, np.int64)
    for g in range(2):
        nodes_g = np.where(grp_of == g)[0]
        og = nodes_g[np.argsort(-deg_own[nodes_g], kind="stable")]
        core_of[og] = _snake(og, NC)

    lo_src = grp_of[src] == 0              # which table half each edge reads

    # --- per (core, group) greedy packing into NTG tiles: keep the heavy
    # (own-group) sum under 5*P and the light sum under 4*P per tile
    slot_of = np.empty(N, np.int64)
    tile_of = np.empty(N, np.int64)        # tile index within the group
    nheav = np.zeros((NC, 2, NTGM), np.int64)
    nlite = np.zeros((NC, 2, NTGM), np.int64)
    cnt_ct = np.zeros((NC, 2, NTGM), np.int64)
    for c in range(NC):
        for g in range(2):
            nodes = np.where((core_of == c) & (grp_of == g))[0]
            nodes = nodes[np.argsort(
                -(deg_own[nodes] * 64 + deg_oth[nodes]), kind="stable")]
            hv = np.zeros(NTGS[g], np.int64)
            lt = np.zeros(NTGS[g], np.int64)
            cnt = np.zeros(NTGS[g], np.int64)
            for v in nodes:
                cost = (hv + deg_own[v]).astype(np.float64) \
                    + 0.02 * (lt + deg_oth[v]) + 1e-4 * cnt \
                    + 1e6 * np.maximum(lt + deg_oth[v] - 4 * P, 0)
                cost[cnt >= P] = 1e18
                t = int(np.argmin(cost))
                tile_of[v] = t
                slot_of[v] = cnt[t]
                cnt[t] += 1
                hv[t] += deg_own[v]
                lt[t] += deg_oth[v]
            nheav[c, g, :NTGS[g]] = hv
            nlite[c, g, :NTGS[g]] = lt
            cnt_ct[c, g, :NTGS[g]] = cnt

    # --- per (core, group) rank permutation to align heavy tiles
    kh_ = (nheav + P - 1) // P
    kl_ = (nlite + P - 1) // P
    perm = np.zeros((NC, 2, NTGM), np.int64)
    for c in range(NC):
        for g in range(2):
            ng = NTGS[g]
            key = (kh_[c, g, :ng] + kl_[c, g, :ng]) + 1e-3 * kh_[c, g, :ng] \
                + 1e-9 * (nheav[c, g, :ng] + nlite[c, g, :ng])
            perm[c, g, :ng] = np.argsort(-key, kind="stable")

    # rank r in [0, NTG) -> group 0, [NTG, NT) -> group 1
    # group 0 tiles: heavy half = lo;  group 1 tiles: heavy half = hi
    Klo = []
    Khi = []
    for r in range(NT):
        g, rr = (0, r) if r < NTG0 else (1, r - NTG0)
        kh = int(max(kh_[c, g, perm[c, g, rr]] for c in range(NC)))
        kl = int(max(kl_[c, g, perm[c, g, rr]] for c in range(NC)))
        if g == 0:
            Klo.append(kh); Khi.append(kl)
        else:
            Klo.append(kl); Khi.append(kh)
    T = [Klo[r] + Khi[r] for r in range(NT)]
    KM = max(max(Klo), max(Khi))

    rank_of = np.zeros((NC, 2, NTGM), np.int64)
    for c in range(NC):
        for g in range(2):
            rank_of[c, g, perm[c, g, :NTGS[g]]] = np.arange(NTGS[g])
    rank_glob = rank_of[core_of, grp_of, tile_of] + grp_of * NTG0
    local_of = rank_glob * P + slot_of
    # table row within the node's half-table: [core][rank-in-group][slot]
    NTG2 = (NTG + 1) // 2
    ntg_of = np.where(grp_of == 0, NTG0, NTG1)
    row_half = core_of * ntg_of * P + rank_of[core_of, grp_of, tile_of] * P \
        + slot_of

    # --- per (core, group, tile) edge lists split by half
    e_core = core_of[dst]
    e_grp = grp_of[dst]
    e_tile = tile_of[dst]
    e_slot = slot_of[dst]
    lists_lo = {}
    lists_hi = {}
    for c in range(NC):
        for g in range(2):
            m_cg = (e_core == c) & (e_grp == g)
            for tl in range(NTGS[g]):
                m = m_cg & (e_tile == tl)
                ml = m & lo_src
                mh = m & ~lo_src
                lists_lo[(c, g, tl)] = (row_half[src[ml]], e_slot[ml])
                lists_hi[(c, g, tl)] = (row_half[src[mh]], e_slot[mh])

    # offsets
    od = np.concatenate([[0], np.cumsum(T)]).astype(np.int64)
    olo = np.concatenate([[0], np.cumsum(Klo)]).astype(np.int64)
    ohi = np.concatenate([[0], np.cumsum(Khi)]).astype(np.int64)
    OD = int(od[-1]); OLO = int(olo[-1]); OHI = int(ohi[-1])

    def pack_idx(flat):
        n = len(flat)
        s = (n + 15) // 16
        arr = np.zeros(s * 16, np.int16)
        arr[:n] = flat
        block = arr.reshape(s, 16).T
        return np.tile(block, (8, 1))

    gidx_lo = np.zeros((NC, P, OLO * 8), np.int16)
    gidx_hi = np.zeros((NC, P, OHI * 8), np.int16)
    drel = np.full((NC, P, OD), -1.0, np.float32)
    for c in range(NC):
        for r in range(NT):
            g, rr = (0, r) if r < NTG0 else (1, r - NTG0)
            tl = perm[c, g, rr]
            for K, off8, dcol0, lst, gax in [
                    (Klo[r], olo[r], od[r], lists_lo[(c, g, tl)], gidx_lo),
                    (Khi[r], ohi[r], od[r] + Klo[r], lists_hi[(c, g, tl)],
                     gidx_hi)]:
                rows, slots = lst
                n = len(rows)
                assert n <= K * P
                flat = np.zeros(K * P, np.int64)
                flat[:n] = rows
                gax[c, :, off8 * 8:(off8 + K) * 8] = pack_idx(flat)
                dr = np.full(K * P, -1.0, np.float32)
                dr[:n] = slots
                drel[c, :, dcol0:dcol0 + K] = dr.reshape(K, P).T

    node_order = np.full((NC, NPC), -1, np.int64)
    for c in range(NC):
        nodes = np.where(core_of == c)[0]
        node_order[c, local_of[nodes]] = nodes

    # per tile col: [ohne (dst-major [d, e]) | ohen (edge-major [e, d])]
    ar = np.arange(P, dtype=np.float32)
    ohne = (ar[None, :, None, None]
            == drel.transpose(0, 2, 1)[:, None, :, :])
    ohen = (drel[:, :, :, None] == ar[None, None, None, :])
    ohb = np.empty((NC, P, 2 * OD * P), ml_dtypes.bfloat16)
    for r in range(NT):
        o0, o1 = int(od[r]), int(od[r + 1])
        t_ = o1 - o0
        ohb[:, :, 2 * o0 * P:(2 * o0 + t_) * P] = \
            ohne[:, :, o0:o1, :].reshape(NC, P, t_ * P)
        ohb[:, :, (2 * o0 + t_) * P:2 * o1 * P] = \
            ohen[:, :, o0:o1, :].reshape(NC, P, t_ * P)

    groups = [(0, NTG0), (NTG0, NT)]

    return dict(NPC=NPC, NT=NT, NTG=NTG, NTG2=NTG2, TBL_G=TBL_G,
                Klo=Klo, Khi=Khi, T=T, KM=KM,
                od=od.tolist(), olo=olo.tolist(), ohi=ohi.tolist(),
                OD=OD, OLO=OLO, OHI=OHI, groups=groups,
                gidx_lo=gidx_lo, gidx_hi=gidx_hi, ohb=ohb,
                node_order=node_order, core_of=core_of, local_of=local_of)


# --------------------------------------------------------------------------
# bass program
# --------------------------------------------------------------------------

def _build_program(dims, post_passes=True):
    PHASES = int(os.environ.get("GAT_PHASES", "3"))
    SHARED = os.environ.get("GAT_SHARED", "1") == "1"
    GB = int(os.environ.get("GAT_GB", "2"))        # gather batch (node tiles)
    SINGLE_PACKET = os.environ.get("GAT_SP", "0") == "1"
    INJ_ACT = os.environ.get("GAT_INJ", "mm") == "act"
    import concourse.bass as bass
    import concourse.mybir as mybir
    import concourse.tile as tile
    from concourse import library_config
    from concourse.bass import _add_dep_helper
    import bass_rust as _br

    fp32 = mybir.dt.float32
    bf = mybir.dt.bfloat16
    i16 = mybir.dt.int16
    AX = mybir.AxisListType
    OP = mybir.AluOpType
    AF = mybir.ActivationFunctionType

    DIN = dims["DIN"]; HC = dims["HC"]; H = dims["H"]; CH = dims["CH"]
    CO = dims["CO"]
    NPC = dims["NPC"]; NT = dims["NT"]; NTG = dims["NTG"]
    NTG2 = dims["NTG2"]
    TBL_G = dims["TBL_G"]
    Klo = dims["Klo"]; Khi = dims["Khi"]; T = dims["T"]; KM = dims["KM"]
    od = dims["od"]; olo = dims["olo"]; ohi = dims["ohi"]
    OD = dims["OD"]; OLO = dims["OLO"]; OHI = dims["OHI"]
    groups = dims["groups"]
    KD = DIN // P
    KH = HC // P
    CO_PAD = 128
    TM = max(T)
    addr_space = "Shared" if SHARED else "Local"

    # gather batches: consecutive ranks within each AG group, <= GB tiles
    batches = []        # (nt0, nt1)
    for g0, g1 in groups:
        nt = g0
        while nt < g1:
            batches.append((nt, min(nt + GB, g1)))
            nt = batches[-1][1]
    BKM = max(max(olo[b1] - olo[b0], ohi[b1] - ohi[b0]) for b0, b1 in batches)
    # phase C uses coarser gather batches (paired within each group) to halve
    # the serial per-gather fixed cost on the Q7
    batchesC = []
    for g0_, g1_ in groups:
        gbs = [b for b in batches if g0_ <= b[0] < g1_]
        i = 0
        while i < len(gbs):
            if i + 1 < len(gbs):
                batchesC.append((gbs[i][0], gbs[i + 1][1]))
                i += 2
            else:
                batchesC.append(gbs[i])
                i += 1
    BKMC = max(max(olo[b1] - olo[b0], ohi[b1] - ohi[b0]) for b0, b1 in batchesC)

    nc = bass.Bass(num_devices=NC, num_swdge_queues=4,
                   dynamic_dma_scratch_size=int(os.environ.get("GAT_DDS", "16384")))

    xkT_d = nc.dram_tensor("xkT", [P, NT * KD * P], bf, kind="ExternalInput")
    w1_d = nc.dram_tensor("w1", [DIN, 2 * HC], bf, kind="ExternalInput")
    w2_d = nc.dram_tensor("w2", [HC, 2 * CO], bf, kind="ExternalInput")
    CCOLS = KM * HC + HC + KM * CO + CO + P
    consts = nc.dram_tensor("consts", [P, CCOLS], bf, kind="ExternalInput")
    constf = nc.dram_tensor("constf", [P, 1], fp32, kind="ExternalInput")
    gidx_lo_d = nc.dram_tensor("gidx_lo", [P, OLO * 8], i16, kind="ExternalInput")
    gidx_hi_d = nc.dram_tensor("gidx_hi", [P, OHI * 8], i16, kind="ExternalInput")
    ohb_d = nc.dram_tensor("ohb", [P, 2 * OD * P], bf, kind="ExternalInput")
    h2_out = nc.dram_tensor("h2o", [NPC, CO], fp32, kind="ExternalOutput")
    ls_out = nc.dram_tensor("lso", [NPC, CO], fp32, kind="ExternalOutput")

    with tile.TileContext(nc) as tc:
        with (
            tc.tile_pool(name="dram", bufs=1, space="DRAM") as dram,
            tc.tile_pool(name="cst", bufs=1) as cst,
        ):
            lib = nc.gpsimd.load_library(library_config.mlp)
            regs = {}
            for b0, b1 in batches + batchesC:
                for n in (olo[b1] - olo[b0], ohi[b1] - ohi[b0]):
                    if n not in regs:
                        regs[n] = nc.gpsimd.to_reg(n * P)

            ctile = cst.tile([P, CCOLS], bf)
            nc.sync.dma_start(out=ctile[:], in_=consts[:])
            cftile = cst.tile([P, 1], fp32)
            nc.sync.dma_start(out=cftile[:], in_=constf[:])
            o = 0
            ident = ctile[:, o:o + P]; o += P
            attBK = ctile[:, o:o + KM * HC]; o += KM * HC
            b1B = ctile[:, o:o + HC]; o += HC
            att2BK = ctile[:, o:o + KM * CO]; o += KM * CO
            b2B = ctile[:, o:o + CO]; o += CO
            alpha = cftile[:, 0:1]

            w1_sb = cst.tile([P, KD, 2 * HC], bf)
            nc.sync.dma_start(out=w1_sb[:], in_=w1_d.rearrange("(k p) c -> p k c", p=P))
            w2_sb = cst.tile([P, KH, 2 * CO], bf)
            nc.sync.dma_start(out=w2_sb[:], in_=w2_d.rearrange("(k p) c -> p k c", p=P))

            gidx_lo_sb = cst.tile([P, OLO * 8], i16)
            nc.sync.dma_start(out=gidx_lo_sb[:], in_=gidx_lo_d[:])
            gidx_hi_sb = cst.tile([P, OHI * 8], i16)
            nc.sync.dma_start(out=gidx_hi_sb[:], in_=gidx_hi_d[:])

            xr1_all = cst.tile([P, NT, HC], bf)
            xr2_all = cst.tile([P, NT, CO], bf)
            xl1_all = cst.tile([P, NT, HC], bf)
            xl2_all = cst.tile([P, NT, CO], bf)
            nc.vector.memset(xr2_all[:], 0.0)

            tbl1 = {}
            tbl2 = {}
            ag1_in = {}
            ag2_in = {}
            for gi, (g0, g1) in enumerate(groups):
                rows = (g1 - g0) * P
                ag1_in[gi] = dram.tile([rows, HC], bf, name=f"ag1i_{gi}")
                ag2_in[gi] = dram.tile([rows, CO_PAD], bf, name=f"ag2i_{gi}")
                tbl1[gi] = dram.tile([NC * rows, HC], bf,
                                     addr_space=addr_space, name=f"tbl1_{gi}")
                tbl2[gi] = dram.tile([NC * rows, CO_PAD], bf,
                                     addr_space=addr_space, name=f"tbl2_{gi}")

            def ag_chunk(src, dst):
                nc.gpsimd.collective_compute(
                    "AllGather", mybir.AluOpType.bypass,
                    replica_groups=[list(range(NC))],
                    ins=[src.opt()],
                    outs=[dst.opt()],
                )



            # ============ phase A: layer-1 projections ============
            with (tc.tile_pool(name="sbA", bufs=3) as sb,
                  tc.tile_pool(name="psA", bufs=2, space="PSUM") as ps):
                XB = 4
                for gi, (g0, g1) in enumerate(groups):
                    for nb in range(g0, g1, XB):
                        ne = min(nb + XB, g1)
                        xt = sb.tile([P, XB, KD, P], bf, tag="xt")
                        nc.sync.dma_start(
                            out=xt[:, 0:ne - nb, :, :],
                            in_=xkT_d[:, nb * KD * P:ne * KD * P])
                        for nt in range(nb, ne):
                            xlr_ps = ps.tile([P, 2 * HC], fp32, tag="mm",
                                             space="PSUM")
                            for k in range(KD):
                                nc.tensor.matmul(out=xlr_ps[:],
                                                 lhsT=xt[:, nt - nb, k, :],
                                                 rhs=w1_sb[:, k, :],
                                                 start=(k == 0),
                                                 stop=(k == KD - 1))
                            nc.scalar.activation(out=xl1_all[:, nt, :],
                                                 in_=xlr_ps[:, 0:HC],
                                                 func=AF.Copy)
                            nc.vector.tensor_copy(out=xr1_all[:, nt, :],
                                                  in_=xlr_ps[:, HC:2 * HC])
                            nc.sync.dma_start(
                                out=ag1_in[gi][(nt - g0) * P:
                                               (nt - g0 + 1) * P, :],
                                in_=xl1_all[:, nt, :])
                    if PHASES >= 2:
                        ag_chunk(ag1_in[gi][:], tbl1[gi][:])

            # ============ phase B: layer-1 edges ============
            if PHASES >= 2:
                grp_of_nt = {}
                for gi, (g0, g1) in enumerate(groups):
                    for nt in range(g0, g1):
                        grp_of_nt[nt] = gi
                with (tc.tile_pool(name="sbB", bufs=2) as sb,
                      tc.tile_pool(name="gbB", bufs=3) as gb,
                      tc.tile_pool(name="psB", bufs=2, space="PSUM") as ps):
                    PF = 8

                    def issue_lo1(i):
                        b0, b1 = batches[i]
                        nlo_b = olo[b1] - olo[b0]
                        glo = gb.tile([P, BKM, HC], bf, tag="glo",
                                      bufs=PF + 2)
                        gi_ = nc.gpsimd.dma_gather(
                            glo[:, 0:nlo_b, :], tbl1[0][:],
                            gidx_lo_sb[:, olo[b0] * 8:olo[b1] * 8],
                            nlo_b * P, regs[nlo_b], HC,
                            queue_num=1, single_packet=SINGLE_PACKET)
                        _add_dep_helper(gi_.ins, lib.ins, sync=False,
                                        reason="lib")
                        return glo

                    lo_pend = {}
                    for i in range(min(PF, len(batches))):
                        lo_pend[i] = issue_lo1(i)
                    for bi, (b0, b1) in enumerate(batches):
                        nlo_b = olo[b1] - olo[b0]
                        nhi_b = ohi[b1] - ohi[b0]
                        glo = lo_pend.pop(bi)
                        ghi = gb.tile([P, BKM, HC], bf, tag="ghi")
                        g2i = nc.gpsimd.dma_gather(
                            ghi[:, 0:nhi_b, :], tbl1[1][:],
                            gidx_hi_sb[:, ohi[b0] * 8:ohi[b1] * 8],
                            nhi_b * P, regs[nhi_b], HC,
                            queue_num=0, single_packet=SINGLE_PACKET)
                        _add_dep_helper(g2i.ins, lib.ins, sync=False,
                                        reason="lib")
                        if bi + PF < len(batches):
                            lo_pend[bi + PF] = issue_lo1(bi + PF)

                        for nt in range(b0, b1):
                            T_ = T[nt]; Klo_ = Klo[nt]; Khi_ = Khi[nt]
                            blo = olo[nt] - olo[b0]
                            bhi = ohi[nt] - ohi[b0]
                            ohb_t = sb.tile([P, 2 * TM * P], bf, tag="ohb")
                            nc.sync.dma_start(
                                out=ohb_t[:, 0:2 * T_ * P],
                                in_=ohb_d[:, 2 * od[nt] * P:2 * (od[nt] + T_) * P])
                            ohne = ohb_t[:, 0:T_ * P]
                            ohen = ohb_t[:, T_ * P:2 * T_ * P]

                            acc = ps.tile([P, HC + H], fp32, tag="acc",
                                          space="PSUM", bufs=2)
                            msg = sb.tile([P, TM, HC + H], bf, tag="msg")
                            # slot-aligned self-loop tile: z = xl + xr, scatter
                            # via identity (also keeps pad-slot denominators
                            # positive, so no NaNs on padding)
                            zs = sb.tile([P, HC], bf, tag="zs")
                            nc.vector.tensor_tensor(out=zs[:],
                                                    in0=xl1_all[:, nt, :],
                                                    in1=xr1_all[:, nt, :],
                                                    op=OP.add)
                            ts_s = sb.tile([P, HC], bf, tag="ts_s")
                            nc.scalar.activation(out=ts_s[:], in_=zs[:],
                                                 func=AF.Prelu, alpha=alpha)
                            tas = sb.tile([P, HC], bf, tag="tas")
                            nc.vector.tensor_tensor(out=tas[:], in0=ts_s[:],
                                                    in1=attBK[:, 0:HC],
                                                    op=OP.mult)
                            msgs = sb.tile([P, HC + H], bf, tag="msgs")
                            scs = sb.tile([P, H], fp32, tag="scs")
                            nc.vector.tensor_reduce(
                                out=scs[:],
                                in_=tas[:].rearrange("p (h c) -> p h c", h=H),
                                axis=AX.X, op=OP.add)
                            nc.scalar.activation(out=msgs[:, HC:HC + H],
                                                 in_=scs[:], func=AF.Exp)
                            nc.vector.tensor_tensor(
                                out=msgs[:, 0:HC].rearrange(
                                    "p (h c) -> p h c", h=H),
                                in0=xl1_all[:, nt, :].rearrange(
                                    "p (h c) -> p h c", h=H),
                                in1=msgs[:, HC:HC + H][:, :, None]
                                    .to_broadcast([P, H, CH]),
                                op=OP.mult)
                            nc.tensor.matmul(out=acc[:], lhsT=ident,
                                             rhs=msgs[:],
                                             start=True, stop=(T_ == 0))
                            for t0, K, gx, gb0 in [(0, Klo_, glo, blo),
                                                   (Klo_, Khi_, ghi, bhi)]:
                                if K == 0:
                                    continue
                                t_sb = sb.tile([P, KM, HC], bf, tag="t")
                                for jj in range(0, K, 3):
                                    je = min(jj + 3, K)
                                    zc = ps.tile([P, 3, HC], fp32, tag="z",
                                                 space="PSUM", bufs=2)
                                    for j in range(jj, je):
                                        nc.tensor.matmul(
                                            out=zc[:, j - jj, :],
                                            lhsT=ohne[:, (t0 + j) * P:(t0 + j + 1) * P],
                                            rhs=xr1_all[:, nt, :],
                                            start=True, stop=False)
                                        nc.tensor.matmul(
                                            out=zc[:, j - jj, :], lhsT=ident,
                                            rhs=gx[:, gb0 + j, :],
                                            start=False, stop=True)
                                    nc.scalar.activation(out=t_sb[:, jj:je, :],
                                                         in_=zc[:, 0:je - jj, :],
                                                         func=AF.Prelu, alpha=alpha)
                                ta = sb.tile([P, KM, HC], bf, tag="ta")
                                nc.vector.tensor_tensor(
                                    out=ta[:, 0:K, :], in0=t_sb[:, 0:K, :],
                                    in1=attBK[:, 0:K * HC].rearrange(
                                        "p (k c) -> p k c", k=K), op=OP.mult)
                                sc = sb.tile([P, KM * H], fp32, tag="sc")
                                nc.vector.tensor_reduce(
                                    out=sc[:, 0:K * H],
                                    in_=ta[:, 0:K, :].rearrange(
                                        "p k (h c) -> p (k h) c", h=H),
                                    axis=AX.X, op=OP.add)
                                nc.scalar.activation(
                                    out=msg[:, t0:t0 + K, HC:HC + H],
                                    in_=sc[:, 0:K * H], func=AF.Exp)
                                nc.vector.tensor_tensor(
                                    out=msg[:, t0:t0 + K, 0:HC].rearrange(
                                        "p k (h c) -> p k h c", h=H),
                                    in0=gx[:, gb0:gb0 + K, :].rearrange(
                                        "p k (h c) -> p k h c", h=H),
                                    in1=msg[:, t0:t0 + K, HC:HC + H]
                                        [:, :, :, None]
                                        .to_broadcast([P, K, H, CH]),
                                    op=OP.mult)
                                for j in range(K):
                                    nc.tensor.matmul(
                                        out=acc[:],
                                        lhsT=ohen[:, (t0 + j) * P:(t0 + j + 1) * P],
                                        rhs=msg[:, t0 + j, :],
                                        start=False, stop=(t0 + j == T_ - 1))

                            rec = sb.tile([P, H], fp32, tag="rec")
                            nc.vector.reciprocal(out=rec[:], in_=acc[:, HC:HC + H])
                            h1 = sb.tile([P, HC], fp32, tag="h1")
                            nc.vector.tensor_tensor(
                                out=h1[:].rearrange("p (h c) -> p h c", h=H),
                                in0=acc[:, 0:HC].rearrange("p (h c) -> p h c", h=H),
                                in1=rec[:, :, None].to_broadcast([P, H, CH]),
                                op=OP.mult)
                            if dims["add_b1"]:
                                nc.vector.tensor_tensor(out=h1[:], in0=h1[:],
                                                        in1=b1B, op=OP.add)
                            # elu(x) = relu(x) - relu(1 - exp(x))
                            eh = sb.tile([P, HC], fp32, tag="eh")
                            nc.scalar.activation(out=eh[:], in_=h1[:], func=AF.Exp)
                            rn = sb.tile([P, HC], fp32, tag="em")
                            nc.scalar.activation(out=rn[:], in_=eh[:], func=AF.Relu,
                                                 scale=-1.0, bias=1.0)
                            rh = sb.tile([P, HC], fp32, tag="rh")
                            nc.scalar.activation(out=rh[:], in_=h1[:], func=AF.Relu)
                            elu = sb.tile([P, HC], bf, tag="elu")
                            nc.vector.tensor_tensor(out=elu[:], in0=rh[:],
                                                    in1=rn[:], op=OP.subtract)

                            tail_ps = ps.tile([P, KH * P + 2 * CO], fp32,
                                              tag="tail", space="PSUM", bufs=2)
                            for k in range(KH):
                                nc.tensor.matmul(
                                    out=tail_ps[:, k * P:(k + 1) * P],
                                    lhsT=elu[:, k * P:(k + 1) * P],
                                    rhs=ident, start=True, stop=True)
                            hT_sb = sb.tile([P, KH, P], bf, tag="hTs")
                            nc.scalar.activation(
                                out=hT_sb[:],
                                in_=tail_ps[:, 0:KH * P].rearrange(
                                    "p (k q) -> p k q", k=KH),
                                func=AF.Copy)
                            x2_ps = tail_ps[:, KH * P:KH * P + 2 * CO]
                            for k in range(KH):
                                nc.tensor.matmul(out=x2_ps, lhsT=hT_sb[:, k, :],
                                                 rhs=w2_sb[:, k, :],
                                                 start=(k == 0), stop=(k == KH - 1))
                            nc.scalar.activation(out=xl2_all[:, nt, :],
                                                 in_=x2_ps[:, 0:CO],
                                                 func=AF.Copy)
                            nc.vector.tensor_copy(out=xr2_all[:, nt, :],
                                                  in_=x2_ps[:, CO:2 * CO])
                            gi = grp_of_nt[nt]
                            g0_, _ = groups[gi]
                            nc.sync.dma_start(
                                out=ag2_in[gi][(nt - g0_) * P:(nt - g0_ + 1) * P,
                                               0:CO],
                                in_=xl2_all[:, nt, :])
                        if PHASES >= 3 and b1 == groups[grp_of_nt[b0]][1]:
                            gi = grp_of_nt[b0]
                            g0_, g1_ = groups[gi]
                            ag_chunk(ag2_in[gi][:], tbl2[gi][:])

            # ============ phase C: layer-2 edges ============
            if PHASES >= 3:
                with (tc.tile_pool(name="sbC", bufs=2) as sb,
                      tc.tile_pool(name="gbC", bufs=3) as gb,
                      tc.tile_pool(name="psC", bufs=2, space="PSUM") as ps):
                    PF = 6

                    def issue_lo2(i):
                        b0, b1 = batchesC[i]
                        nlo_b = olo[b1] - olo[b0]
                        g2lo = gb.tile([P, BKMC, CO_PAD], bf, tag="g2lo",
                                       bufs=PF + 2)
                        gi_ = nc.gpsimd.dma_gather(
                            g2lo[:, 0:nlo_b, :], tbl2[0][:],
                            gidx_lo_sb[:, olo[b0] * 8:olo[b1] * 8],
                            nlo_b * P, regs[nlo_b], CO_PAD,
                            queue_num=1, single_packet=SINGLE_PACKET)
                        _add_dep_helper(gi_.ins, lib.ins, sync=False,
                                        reason="lib")
                        return g2lo

                    lo_pend = {}
                    for i in range(min(PF, len(batchesC))):
                        lo_pend[i] = issue_lo2(i)
                    for bi, (b0, b1) in enumerate(batchesC):
                        nlo_b = olo[b1] - olo[b0]
                        nhi_b = ohi[b1] - ohi[b0]
                        g2lo = lo_pend.pop(bi)
                        g2hi = gb.tile([P, BKMC, CO_PAD], bf, tag="g2hi")
                        g2i = nc.gpsimd.dma_gather(
                            g2hi[:, 0:nhi_b, :], tbl2[1][:],
                            gidx_hi_sb[:, ohi[b0] * 8:ohi[b1] * 8],
                            nhi_b * P, regs[nhi_b], CO_PAD,
                            queue_num=0, single_packet=SINGLE_PACKET)
                        _add_dep_helper(g2i.ins, lib.ins, sync=False,
                                        reason="lib")
                        if bi + PF < len(batchesC):
                            lo_pend[bi + PF] = issue_lo2(bi + PF)

                        for nt in range(b0, b1):
                            T_ = T[nt]; Klo_ = Klo[nt]; Khi_ = Khi[nt]
                            blo = olo[nt] - olo[b0]
                            bhi = ohi[nt] - ohi[b0]
                            ohb_t = sb.tile([P, 2 * TM * P], bf, tag="ohb2")
                            nc.sync.dma_start(
                                out=ohb_t[:, 0:2 * T_ * P],
                                in_=ohb_d[:, 2 * od[nt] * P:2 * (od[nt] + T_) * P])
                            ohne = ohb_t[:, 0:T_ * P]
                            ohen = ohb_t[:, T_ * P:2 * T_ * P]

                            acc2 = ps.tile([P, CO + 1], fp32, tag="acc2",
                                           space="PSUM", bufs=2)
                            msg2 = sb.tile([P, TM, CO + 1], bf, tag="msg2")
                            # self-loop tile (see phase B)
                            z2s = sb.tile([P, CO], bf, tag="z2s")
                            nc.vector.tensor_tensor(out=z2s[:],
                                                    in0=xl2_all[:, nt, :],
                                                    in1=xr2_all[:, nt, :],
                                                    op=OP.add)
                            t2s = sb.tile([P, CO], bf, tag="t2s")
                            nc.scalar.activation(out=t2s[:], in_=z2s[:],
                                                 func=AF.Prelu, alpha=alpha)
                            ta2s = sb.tile([P, CO], bf, tag="ta2s")
                            nc.vector.tensor_tensor(out=ta2s[:], in0=t2s[:],
                                                    in1=att2BK[:, 0:CO],
                                                    op=OP.mult)
                            msg2s = sb.tile([P, CO + 1], bf, tag="msg2s")
                            sc2s = sb.tile([P, 1], fp32, tag="sc2s")
                            nc.vector.tensor_reduce(out=sc2s[:], in_=ta2s[:],
                                                    axis=AX.X, op=OP.add)
                            nc.scalar.activation(out=msg2s[:, CO:CO + 1],
                                                 in_=sc2s[:], func=AF.Exp)
                            nc.vector.tensor_tensor(
                                out=msg2s[:, 0:CO], in0=xl2_all[:, nt, :],
                                in1=msg2s[:, CO:CO + 1].to_broadcast([P, CO]),
                                op=OP.mult)
                            nc.tensor.matmul(out=acc2[:], lhsT=ident,
                                             rhs=msg2s[:],
                                             start=True, stop=(T_ == 0))
                            for t0, K, gx, gb0 in [(0, Klo_, g2lo, blo),
                                                   (Klo_, Khi_, g2hi, bhi)]:
                                if K == 0:
                                    continue
                                t2 = sb.tile([P, KM, CO], bf, tag="t2")
                                zc = ps.tile([P, KM, CO], fp32, tag="z2",
                                             space="PSUM", bufs=2)
                                for j in range(K):
                                    nc.tensor.matmul(
                                        out=zc[:, j, :],
                                        lhsT=ohne[:, (t0 + j) * P:(t0 + j + 1) * P],
                                        rhs=xr2_all[:, nt, :],
                                        start=True, stop=False)
                                    nc.tensor.matmul(
                                        out=zc[:, j, :], lhsT=ident,
                                        rhs=gx[:, gb0 + j, 0:CO],
                                        start=False, stop=True)
                                nc.scalar.activation(out=t2[:, 0:K, :],
                                                     in_=zc[:, 0:K, :],
                                                     func=AF.Prelu, alpha=alpha)
                                ta2 = sb.tile([P, KM, CO], bf, tag="ta2")
                                nc.vector.tensor_tensor(
                                    out=ta2[:, 0:K, :], in0=t2[:, 0:K, :],
                                    in1=att2BK[:, 0:K * CO].rearrange(
                                        "p (k c) -> p k c", k=K), op=OP.mult)
                                sc2 = sb.tile([P, KM], fp32, tag="sc2")
                                nc.vector.tensor_reduce(
                                    out=sc2[:, 0:K], in_=ta2[:, 0:K, :],
                                    axis=AX.X, op=OP.add)
                                nc.scalar.activation(
                                    out=msg2[:, t0:t0 + K, CO:CO + 1],
                                    in_=sc2[:, 0:K], func=AF.Exp)
                                nc.vector.tensor_tensor(
                                    out=msg2[:, t0:t0 + K, 0:CO],
                                    in0=gx[:, gb0:gb0 + K, 0:CO],
                                    in1=msg2[:, t0:t0 + K, CO:CO + 1]
                                        .to_broadcast([P, K, CO]),
                                    op=OP.mult)
                                for j in range(K):
                                    nc.tensor.matmul(
                                        out=acc2[:],
                                        lhsT=ohen[:, (t0 + j) * P:(t0 + j + 1) * P],
                                        rhs=msg2[:, t0 + j, :],
                                        start=False, stop=(t0 + j == T_ - 1))

                            rec2 = sb.tile([P, 1], fp32, tag="rec2")
                            nc.vector.reciprocal(out=rec2[:], in_=acc2[:, CO:CO + 1])
                            h2 = sb.tile([P, CO], fp32, tag="h2")
                            nc.scalar.activation(out=h2[:], in_=acc2[:, 0:CO],
                                                 func=AF.Copy,
                                                 scale=rec2[:, 0:1])
                            if dims["add_b2"]:
                                nc.vector.tensor_tensor(out=h2[:], in0=h2[:],
                                                        in1=b2B, op=OP.add)
                            nc.sync.dma_start(
                                out=h2_out.rearrange("(a p) d -> p a d", p=P)
                                    [:, nt, :],
                                in_=h2[:])
                            # h2 is O(1): skip max-subtraction in log_softmax
                            esc = sb.tile([P, CO], fp32, tag="esc")
                            ssum = sb.tile([P, 1], fp32, tag="ssum")
                            nc.scalar.activation(out=esc[:], in_=h2[:], func=AF.Exp,
                                                 accum_out=ssum[:, 0:1])
                            lns = sb.tile([P, 1], fp32, tag="lns")
                            nc.scalar.activation(out=lns[:], in_=ssum[:], func=AF.Ln)
                            ls = sb.tile([P, CO], fp32, tag="ls")
                            nc.vector.tensor_tensor(
                                out=ls[:], in0=h2[:],
                                in1=lns[:, 0:1].to_broadcast([P, CO]),
                                op=OP.subtract)
                            nc.sync.dma_start(
                                out=ls_out.rearrange("(a p) d -> p a d", p=P)
                                    [:, nt, :],
                                in_=ls[:])



    if post_passes:
        _br.generate_event_semaphores(nc)
        _br.codegen_inst_isa_subclasses(nc)
    return nc


# --------------------------------------------------------------------------
# entry point
# --------------------------------------------------------------------------

def kernel(x, edge_index, W1l, W1r, att1, b1, W2l, W2r, att2, b2):
    x = np.asarray(x, np.float32)
    edge_index = np.asarray(edge_index)
    W1l = np.asarray(W1l, np.float32); W1r = np.asarray(W1r, np.float32)
    att1 = np.asarray(att1, np.float32); b1 = np.asarray(b1, np.float32)
    W2l = np.asarray(W2l, np.float32); W2r = np.asarray(W2r, np.float32)
    att2 = np.asarray(att2, np.float32); b2 = np.asarray(b2, np.float32)

    N, DIN = x.shape
    E = edge_index.shape[1]
    H, CH = att1.shape
    HC = W1l.shape[1]
    CO = W2l.shape[1]

    key = (N, E, DIN, H, CH, HC, CO,
           int(np.abs(b1).max() > 0), int(np.abs(b2).max() > 0),
           hash(edge_index.tobytes()))
    if key in _plan_cache:
        pp, nc, dims = _plan_cache[key]
    else:
        pp = _preprocess(N, E, edge_index)
        dims = dict(DIN=DIN, HC=HC, H=H, CH=CH, CO=CO,
                    NPC=pp["NPC"], NT=pp["NT"], NTG=pp["NTG"],
                    NTG2=pp["NTG2"], TBL_G=pp["TBL_G"],
                    Klo=pp["Klo"], Khi=pp["Khi"], T=pp["T"], KM=pp["KM"],
                    od=pp["od"], olo=pp["olo"], ohi=pp["ohi"],
                    OD=pp["OD"], OLO=pp["OLO"], OHI=pp["OHI"],
                    groups=pp["groups"],
                    add_b1=bool(np.abs(b1).max() > 0),
                    add_b2=bool(np.abs(b2).max() > 0))
        nc = _build_program(dims)
        _plan_cache[key] = (pp, nc, dims)

    NPC = pp["NPC"]; NT = pp["NT"]; KM = pp["KM"]
    KD = DIN // P
    bfdt = ml_dtypes.bfloat16

    # consts blob: ident | attBK | b1B | att2BK | b2B
    ident = np.eye(P, dtype=np.float32)
    attBK = np.broadcast_to(
        np.tile(att1.reshape(HC), KM)[None, :], (P, KM * HC))
    b1B = np.broadcast_to(b1.reshape(1, HC), (P, HC))
    att2BK = np.broadcast_to(
        np.tile(att2.reshape(CO), KM)[None, :], (P, KM * CO))
    b2B = np.broadcast_to(b2.reshape(1, CO), (P, CO))
    consts = np.concatenate([ident, attBK, b1B, att2BK, b2B],
                            axis=1).astype(bfdt)
    constf = np.full((P, 1), NEG_SLOPE, np.float32)
    w1cat = np.concatenate([W1l, W1r], axis=1).astype(bfdt)
    w2cat = np.concatenate([W2l, W2r], axis=1).astype(bfdt)

    in_maps = []
    for c in range(NC):
        xkc = np.zeros((NPC, DIN), np.float32)
        sel = pp["node_order"][c]
        real = sel >= 0
        xkc[real] = x[sel[real]]
        # [p, nt, k, q] = xkc[nt*P+q, k*P+p]
        xkT = np.ascontiguousarray(
            xkc.reshape(NT, P, KD, P).transpose(3, 0, 2, 1)
        ).reshape(P, NT * KD * P).astype(bfdt)
        in_maps.append(dict(
            xkT=xkT, w1=w1cat, w2=w2cat, consts=consts, constf=constf,
            gidx_lo=np.ascontiguousarray(pp["gidx_lo"][c]),
            gidx_hi=np.ascontiguousarray(pp["gidx_hi"][c]),
            ohb=np.ascontiguousarray(pp["ohb"][c]),
        ))

    from concourse.bass_utils import run_bass_kernel_spmd
    res = run_bass_kernel_spmd(nc, in_maps, core_ids=list(range(NC)))

    h = np.empty((N, CO), np.float32)
    ls = np.empty((N, CO), np.float32)
    r_core = pp["core_of"]
    r_loc = pp["local_of"]
    for c in range(NC):
        m = r_core == c
        h[m] = res.results[c]["h2o"][r_loc[m]]
        ls[m] = res.results[c]["lso"][r_loc[m]]
    return h, ls



# revision 74
# speedup vs baseline: 1.0157x; 1.0157x over previous
"""2-layer GATv2 (PyG GATv2Conv semantics) on 8 Trainium2 NeuronCores.

Strategy (v3):
  - Nodes sharded across 8 cores; per-core greedy 2D packing balances each
    destination tile's lo/hi in-edge counts (lo/hi = source node group, one
    AllGather chunk / int16-indexable table half each).
  - x is shipped pre-transposed; layer-1 projections are 6 accumulating
    matmuls per node tile with a combined [W1l|W1r] moving operand.
  - xl tables AllGathered HBM->HBM per group, overlapped with compute.
  - Per destination tile, incoming-edge source rows are fetched with
    dma_gather (int16 idx; two table halves; GB node tiles per call).
    SWDGE descriptor generation is serial on the Q7 at ~9ns/row and is the
    hard floor of this design -- hence self-loops are NOT gathered: each
    node tile has a dedicated slot-aligned self tile computed from resident
    xl/xr SBUF copies (DVE add + Prelu + identity-scatter), which also keeps
    pad-slot denominators positive (no NaNs).  Both one-hots (oh_ne dst
    -major, oh_en edge-major) are precomputed on host, streamed from HBM.
  - Per half (K edge tiles): K z-matmul pairs (one-hot xr broadcast +
    identity gx inject) into a shared PSUM chunk, ONE batched Prelu per <=3
    tiles, batched att mult / reduce / exp / msg mult over the half, then K
    scatter matmuls accumulate numerator+denominator in PSUM.
  - elu(x) = relu(x) - relu(1 - exp(x)): 3 ACT ops + 1 DVE op.
  - Softmax skips max-subtraction (scores O(1)); log_softmax likewise.
  - Layer 2 (heads=1, 16 ch) repeats the edge structure on a 256B-row
    table; epilogue division/log-softmax moved to ACT (scale/bias) where
    possible.

kernel(**inputs) takes FULL inputs, returns FULL outputs.
"""

import os
import sys

if "/opt/trn_rl_repo" not in sys.path:
    sys.path.insert(0, "/opt/trn_rl_repo")

import numpy as np
import ml_dtypes

NC = 8          # cores
P = 128         # partitions
NEG_SLOPE = 0.2
NGRP = 2        # AllGather chunks

_plan_cache = {}


# --------------------------------------------------------------------------
# host-side graph preprocessing
# --------------------------------------------------------------------------

def _snake(order, nbins):
    n = len(order)
    ids = np.arange(n)
    round_ = ids // nbins
    pos = ids % nbins
    b = np.where(round_ % 2 == 0, pos, nbins - 1 - pos)
    out = np.empty(n, np.int64)
    out[:] = b
    return out


def _preprocess(N, E, edge_index):
    # Two rank-groups (= AllGather chunks = table halves), 25 tile-ranks each.
    # (An unequal 40/60 split was tried to start the gather stream earlier; it
    # is 270us WORSE: the bigger second AllGather delays the hi-half tables
    # that phase B consumes tile-by-tile. Equal halves are optimal.)
    NTG = ((N + 2 * NC - 1) // (2 * NC) + P - 1) // P      # tiles per group
    NT = 2 * NTG
    NPC = NT * P
    NTG0 = NT // 2
    NTG1 = NT - NTG0
    NTGS = (NTG0, NTG1)
    NTGM = max(NTGS)
    TBL_G = NC * NTGM * P                                  # rows per half-table
    assert TBL_G < 32768, "table half must fit int16 row indices"

    # self-loops are handled by a dedicated slot-aligned "self tile" per node
    # tile in the kernel (no gather needed) -- only real edges go in lists.
    src = edge_index[0].astype(np.int64)
    dst = edge_index[1].astype(np.int64)
    deg = np.bincount(dst, minlength=N)

    # --- group assignment (fixes each edge's table half), then cores within
    # each group balanced by OWN-group in-degree (the heavy, self-loop half)
    order = np.argsort(-deg, kind="stable")
    grp_of = np.empty(N, np.int64)
    grp_of[order] = _snake(order, 2)
    e_own = grp_of[src] == grp_of[dst]
    deg_own = np.bincount(dst[e_own], minlength=N)
    deg_oth = deg - deg_own
    core_of = np.empty(N, np.int64)
    for g in range(2):
        nodes_g = np.where(grp_of == g)[0]
        og = nodes_g[np.argsort(-deg_own[nodes_g], kind="stable")]
        core_of[og] = _snake(og, NC)

    lo_src = grp_of[src] == 0              # which table half each edge reads

    # --- per (core, group) greedy packing into NTG tiles: keep the heavy
    # (own-group) sum under 5*P and the light sum under 4*P per tile
    slot_of = np.empty(N, np.int64)
    tile_of = np.empty(N, np.int64)        # tile index within the group
    nheav = np.zeros((NC, 2, NTGM), np.int64)
    nlite = np.zeros((NC, 2, NTGM), np.int64)
    cnt_ct = np.zeros((NC, 2, NTGM), np.int64)
    for c in range(NC):
        for g in range(2):
            nodes = np.where((core_of == c) & (grp_of == g))[0]
            nodes = nodes[np.argsort(
                -(deg_own[nodes] * 64 + deg_oth[nodes]), kind="stable")]
            hv = np.zeros(NTGS[g], np.int64)
            lt = np.zeros(NTGS[g], np.int64)
            cnt = np.zeros(NTGS[g], np.int64)
            for v in nodes:
                cost = (hv + deg_own[v]).astype(np.float64) \
                    + 0.02 * (lt + deg_oth[v]) + 1e-4 * cnt \
                    + 1e6 * np.maximum(lt + deg_oth[v] - 4 * P, 0)
                cost[cnt >= P] = 1e18
                t = int(np.argmin(cost))
                tile_of[v] = t
                slot_of[v] = cnt[t]
                cnt[t] += 1
                hv[t] += deg_own[v]
                lt[t] += deg_oth[v]
            nheav[c, g, :NTGS[g]] = hv
            nlite[c, g, :NTGS[g]] = lt
            cnt_ct[c, g, :NTGS[g]] = cnt

    # --- per (core, group) rank permutation to align heavy tiles
    kh_ = (nheav + P - 1) // P
    kl_ = (nlite + P - 1) // P
    perm = np.zeros((NC, 2, NTGM), np.int64)
    for c in range(NC):
        for g in range(2):
            ng = NTGS[g]
            key = (kh_[c, g, :ng] + kl_[c, g, :ng]) + 1e-3 * kh_[c, g, :ng] \
                + 1e-9 * (nheav[c, g, :ng] + nlite[c, g, :ng])
            perm[c, g, :ng] = np.argsort(-key, kind="stable")

    # rank r in [0, NTG) -> group 0, [NTG, NT) -> group 1
    # group 0 tiles: heavy half = lo;  group 1 tiles: heavy half = hi
    Klo = []
    Khi = []
    for r in range(NT):
        g, rr = (0, r) if r < NTG0 else (1, r - NTG0)
        kh = int(max(kh_[c, g, perm[c, g, rr]] for c in range(NC)))
        kl = int(max(kl_[c, g, perm[c, g, rr]] for c in range(NC)))
        if g == 0:
            Klo.append(kh); Khi.append(kl)
        else:
            Klo.append(kl); Khi.append(kh)
    T = [Klo[r] + Khi[r] for r in range(NT)]
    KM = max(max(Klo), max(Khi))

    rank_of = np.zeros((NC, 2, NTGM), np.int64)
    for c in range(NC):
        for g in range(2):
            rank_of[c, g, perm[c, g, :NTGS[g]]] = np.arange(NTGS[g])
    rank_glob = rank_of[core_of, grp_of, tile_of] + grp_of * NTG0
    local_of = rank_glob * P + slot_of
    # table row within the node's half-table: [core][rank-in-group][slot]
    NTG2 = (NTG + 1) // 2
    ntg_of = np.where(grp_of == 0, NTG0, NTG1)
    row_half = core_of * ntg_of * P + rank_of[core_of, grp_of, tile_of] * P \
        + slot_of

    # --- per (core, group, tile) edge lists split by half
    e_core = core_of[dst]
    e_grp = grp_of[dst]
    e_tile = tile_of[dst]
    e_slot = slot_of[dst]
    lists_lo = {}
    lists_hi = {}
    for c in range(NC):
        for g in range(2):
            m_cg = (e_core == c) & (e_grp == g)
            for tl in range(NTGS[g]):
                m = m_cg & (e_tile == tl)
                ml = m & lo_src
                mh = m & ~lo_src
                lists_lo[(c, g, tl)] = (row_half[src[ml]], e_slot[ml])
                lists_hi[(c, g, tl)] = (row_half[src[mh]], e_slot[mh])

    # offsets
    od = np.concatenate([[0], np.cumsum(T)]).astype(np.int64)
    olo = np.concatenate([[0], np.cumsum(Klo)]).astype(np.int64)
    ohi = np.concatenate([[0], np.cumsum(Khi)]).astype(np.int64)
    OD = int(od[-1]); OLO = int(olo[-1]); OHI = int(ohi[-1])

    def pack_idx(flat):
        n = len(flat)
        s = (n + 15) // 16
        arr = np.zeros(s * 16, np.int16)
        arr[:n] = flat
        block = arr.reshape(s, 16).T
        return np.tile(block, (8, 1))

    gidx_lo = np.zeros((NC, P, OLO * 8), np.int16)
    gidx_hi = np.zeros((NC, P, OHI * 8), np.int16)
    drel = np.full((NC, P, OD), -1.0, np.float32)
    for c in range(NC):
        for r in range(NT):
            g, rr = (0, r) if r < NTG0 else (1, r - NTG0)
            tl = perm[c, g, rr]
            for K, off8, dcol0, lst, gax in [
                    (Klo[r], olo[r], od[r], lists_lo[(c, g, tl)], gidx_lo),
                    (Khi[r], ohi[r], od[r] + Klo[r], lists_hi[(c, g, tl)],
                     gidx_hi)]:
                rows, slots = lst
                n = len(rows)
                assert n <= K * P
                flat = np.zeros(K * P, np.int64)
                flat[:n] = rows
                gax[c, :, off8 * 8:(off8 + K) * 8] = pack_idx(flat)
                dr = np.full(K * P, -1.0, np.float32)
                dr[:n] = slots
                drel[c, :, dcol0:dcol0 + K] = dr.reshape(K, P).T

    node_order = np.full((NC, NPC), -1, np.int64)
    for c in range(NC):
        nodes = np.where(core_of == c)[0]
        node_order[c, local_of[nodes]] = nodes

    # per tile col: [ohne (dst-major [d, e]) | ohen (edge-major [e, d])]
    ar = np.arange(P, dtype=np.float32)
    ohne = (ar[None, :, None, None]
            == drel.transpose(0, 2, 1)[:, None, :, :])
    ohen = (drel[:, :, :, None] == ar[None, None, None, :])
    ohb = np.empty((NC, P, 2 * OD * P), ml_dtypes.bfloat16)
    for r in range(NT):
        o0, o1 = int(od[r]), int(od[r + 1])
        t_ = o1 - o0
        ohb[:, :, 2 * o0 * P:(2 * o0 + t_) * P] = \
            ohne[:, :, o0:o1, :].reshape(NC, P, t_ * P)
        ohb[:, :, (2 * o0 + t_) * P:2 * o1 * P] = \
            ohen[:, :, o0:o1, :].reshape(NC, P, t_ * P)

    groups = [(0, NTG0), (NTG0, NT)]

    return dict(NPC=NPC, NT=NT, NTG=NTG, NTG2=NTG2, TBL_G=TBL_G,
                Klo=Klo, Khi=Khi, T=T, KM=KM,
                od=od.tolist(), olo=olo.tolist(), ohi=ohi.tolist(),
                OD=OD, OLO=OLO, OHI=OHI, groups=groups,
                gidx_lo=gidx_lo, gidx_hi=gidx_hi, ohb=ohb,
                node_order=node_order, core_of=core_of, local_of=local_of)


# --------------------------------------------------------------------------
# bass program
# --------------------------------------------------------------------------

def _build_program(dims, post_passes=True):
    PHASES = int(os.environ.get("GAT_PHASES", "3"))
    SHARED = os.environ.get("GAT_SHARED", "1") == "1"
    GB = int(os.environ.get("GAT_GB", "2"))        # gather batch (node tiles)
    SINGLE_PACKET = os.environ.get("GAT_SP", "0") == "1"
    INJ_ACT = os.environ.get("GAT_INJ", "mm") == "act"
    import concourse.bass as bass
    import concourse.mybir as mybir
    import concourse.tile as tile
    from concourse import library_config
    from concourse.bass import _add_dep_helper
    import bass_rust as _br

    fp32 = mybir.dt.float32
    bf = mybir.dt.bfloat16
    i16 = mybir.dt.int16
    AX = mybir.AxisListType
    OP = mybir.AluOpType
    AF = mybir.ActivationFunctionType

    DIN = dims["DIN"]; HC = dims["HC"]; H = dims["H"]; CH = dims["CH"]
    CO = dims["CO"]
    NPC = dims["NPC"]; NT = dims["NT"]; NTG = dims["NTG"]
    NTG2 = dims["NTG2"]
    TBL_G = dims["TBL_G"]
    Klo = dims["Klo"]; Khi = dims["Khi"]; T = dims["T"]; KM = dims["KM"]
    od = dims["od"]; olo = dims["olo"]; ohi = dims["ohi"]
    OD = dims["OD"]; OLO = dims["OLO"]; OHI = dims["OHI"]
    groups = dims["groups"]
    KD = DIN // P
    KH = HC // P
    CO_PAD = 128
    TM = max(T)
    addr_space = "Shared" if SHARED else "Local"

    # gather batches: consecutive ranks within each AG group, <= GB tiles
    batches = []        # (nt0, nt1)
    for g0, g1 in groups:
        nt = g0
        while nt < g1:
            batches.append((nt, min(nt + GB, g1)))
            nt = batches[-1][1]
    BKM = max(max(olo[b1] - olo[b0], ohi[b1] - ohi[b0]) for b0, b1 in batches)
    # phase C uses coarser gather batches (paired within each group) to halve
    # the serial per-gather fixed cost on the Q7
    batchesC = []
    for g0_, g1_ in groups:
        gbs = [b for b in batches if g0_ <= b[0] < g1_]
        i = 0
        while i < len(gbs):
            if i + 1 < len(gbs):
                batchesC.append((gbs[i][0], gbs[i + 1][1]))
                i += 2
            else:
                batchesC.append(gbs[i])
                i += 1
    BKMC = max(max(olo[b1] - olo[b0], ohi[b1] - ohi[b0]) for b0, b1 in batchesC)

    nc = bass.Bass(num_devices=NC, num_swdge_queues=4,
                   dynamic_dma_scratch_size=int(os.environ.get("GAT_DDS", "16384")))

    xkT_d = nc.dram_tensor("xkT", [P, NT * KD * P], bf, kind="ExternalInput")
    w1_d = nc.dram_tensor("w1", [DIN, 2 * HC], bf, kind="ExternalInput")
    w2_d = nc.dram_tensor("w2", [HC, 2 * CO], bf, kind="ExternalInput")
    CCOLS = KM * HC + HC + KM * CO + CO + P
    consts = nc.dram_tensor("consts", [P, CCOLS], bf, kind="ExternalInput")
    constf = nc.dram_tensor("constf", [P, 1], fp32, kind="ExternalInput")
    gidx_lo_d = nc.dram_tensor("gidx_lo", [P, OLO * 8], i16, kind="ExternalInput")
    gidx_hi_d = nc.dram_tensor("gidx_hi", [P, OHI * 8], i16, kind="ExternalInput")
    ohb_d = nc.dram_tensor("ohb", [P, 2 * OD * P], bf, kind="ExternalInput")
    h2_out = nc.dram_tensor("h2o", [NPC, CO], fp32, kind="ExternalOutput")
    ls_out = nc.dram_tensor("lso", [NPC, CO], fp32, kind="ExternalOutput")

    with tile.TileContext(nc) as tc:
        with (
            tc.tile_pool(name="dram", bufs=1, space="DRAM") as dram,
            tc.tile_pool(name="cst", bufs=1) as cst,
        ):
            lib = nc.gpsimd.load_library(library_config.mlp)
            regs = {}
            for b0, b1 in batches + batchesC:
                for n in (olo[b1] - olo[b0], ohi[b1] - ohi[b0]):
                    if n not in regs:
                        regs[n] = nc.gpsimd.to_reg(n * P)

            ctile = cst.tile([P, CCOLS], bf)
            nc.sync.dma_start(out=ctile[:], in_=consts[:])
            cftile = cst.tile([P, 1], fp32)
            nc.sync.dma_start(out=cftile[:], in_=constf[:])
            o = 0
            ident = ctile[:, o:o + P]; o += P
            attBK = ctile[:, o:o + KM * HC]; o += KM * HC
            b1B = ctile[:, o:o + HC]; o += HC
            att2BK = ctile[:, o:o + KM * CO]; o += KM * CO
            b2B = ctile[:, o:o + CO]; o += CO
            alpha = cftile[:, 0:1]

            w1_sb = cst.tile([P, KD, 2 * HC], bf)
            nc.sync.dma_start(out=w1_sb[:], in_=w1_d.rearrange("(k p) c -> p k c", p=P))
            w2_sb = cst.tile([P, KH, 2 * CO], bf)
            nc.sync.dma_start(out=w2_sb[:], in_=w2_d.rearrange("(k p) c -> p k c", p=P))

            gidx_lo_sb = cst.tile([P, OLO * 8], i16)
            nc.sync.dma_start(out=gidx_lo_sb[:], in_=gidx_lo_d[:])
            gidx_hi_sb = cst.tile([P, OHI * 8], i16)
            nc.sync.dma_start(out=gidx_hi_sb[:], in_=gidx_hi_d[:])

            xr1_all = cst.tile([P, NT, HC], bf)
            xr2_all = cst.tile([P, NT, CO], bf)
            xl1_all = cst.tile([P, NT, HC], bf)
            xl2_all = cst.tile([P, NT, CO], bf)
            nc.vector.memset(xr2_all[:], 0.0)

            tbl1 = {}
            tbl2 = {}
            ag1_in = {}
            ag2_in = {}
            for gi, (g0, g1) in enumerate(groups):
                rows = (g1 - g0) * P
                ag1_in[gi] = dram.tile([rows, HC], bf, name=f"ag1i_{gi}")
                ag2_in[gi] = dram.tile([rows, CO_PAD], bf, name=f"ag2i_{gi}")
                tbl1[gi] = dram.tile([NC * rows, HC], bf,
                                     addr_space=addr_space, name=f"tbl1_{gi}")
                tbl2[gi] = dram.tile([NC * rows, CO_PAD], bf,
                                     addr_space=addr_space, name=f"tbl2_{gi}")

            def ag_chunk(src, dst):
                nc.gpsimd.collective_compute(
                    "AllGather", mybir.AluOpType.bypass,
                    replica_groups=[list(range(NC))],
                    ins=[src.opt()],
                    outs=[dst.opt()],
                )



            # ============ phase A: layer-1 projections ============
            with (tc.tile_pool(name="sbA", bufs=3) as sb,
                  tc.tile_pool(name="psA", bufs=2, space="PSUM") as ps):
                XB = 4
                for gi, (g0, g1) in enumerate(groups):
                    for nb in range(g0, g1, XB):
                        ne = min(nb + XB, g1)
                        xt = sb.tile([P, XB, KD, P], bf, tag="xt")
                        nc.sync.dma_start(
                            out=xt[:, 0:ne - nb, :, :],
                            in_=xkT_d[:, nb * KD * P:ne * KD * P])
                        for nt in range(nb, ne):
                            xlr_ps = ps.tile([P, 2 * HC], fp32, tag="mm",
                                             space="PSUM")
                            for k in range(KD):
                                nc.tensor.matmul(out=xlr_ps[:],
                                                 lhsT=xt[:, nt - nb, k, :],
                                                 rhs=w1_sb[:, k, :],
                                                 start=(k == 0),
                                                 stop=(k == KD - 1))
                            nc.scalar.activation(out=xl1_all[:, nt, :],
                                                 in_=xlr_ps[:, 0:HC],
                                                 func=AF.Copy)
                            nc.vector.tensor_copy(out=xr1_all[:, nt, :],
                                                  in_=xlr_ps[:, HC:2 * HC])
                            nc.sync.dma_start(
                                out=ag1_in[gi][(nt - g0) * P:
                                               (nt - g0 + 1) * P, :],
                                in_=xl1_all[:, nt, :])
                    if PHASES >= 2:
                        ag_chunk(ag1_in[gi][:], tbl1[gi][:])

            # ============ phase B: layer-1 edges ============
            if PHASES >= 2:
                grp_of_nt = {}
                for gi, (g0, g1) in enumerate(groups):
                    for nt in range(g0, g1):
                        grp_of_nt[nt] = gi
                with (tc.tile_pool(name="sbB", bufs=2) as sb,
                      tc.tile_pool(name="gbB", bufs=3) as gb,
                      tc.tile_pool(name="psB", bufs=2, space="PSUM") as ps):
                    PF = 6

                    def issue_lo1(i):
                        b0, b1 = batches[i]
                        nlo_b = olo[b1] - olo[b0]
                        glo = gb.tile([P, BKM, HC], bf, tag="glo",
                                      bufs=PF + 2)
                        gi_ = nc.gpsimd.dma_gather(
                            glo[:, 0:nlo_b, :], tbl1[0][:],
                            gidx_lo_sb[:, olo[b0] * 8:olo[b1] * 8],
                            nlo_b * P, regs[nlo_b], HC,
                            queue_num=1, single_packet=SINGLE_PACKET)
                        _add_dep_helper(gi_.ins, lib.ins, sync=False,
                                        reason="lib")
                        return glo

                    lo_pend = {}
                    for i in range(min(PF, len(batches))):
                        lo_pend[i] = issue_lo1(i)
                    for bi, (b0, b1) in enumerate(batches):
                        nlo_b = olo[b1] - olo[b0]
                        nhi_b = ohi[b1] - ohi[b0]
                        glo = lo_pend.pop(bi)
                        ghi = gb.tile([P, BKM, HC], bf, tag="ghi")
                        g2i = nc.gpsimd.dma_gather(
                            ghi[:, 0:nhi_b, :], tbl1[1][:],
                            gidx_hi_sb[:, ohi[b0] * 8:ohi[b1] * 8],
                            nhi_b * P, regs[nhi_b], HC,
                            queue_num=0, single_packet=SINGLE_PACKET)
                        _add_dep_helper(g2i.ins, lib.ins, sync=False,
                                        reason="lib")
                        if bi + PF < len(batches):
                            lo_pend[bi + PF] = issue_lo1(bi + PF)

                        for nt in range(b0, b1):
                            T_ = T[nt]; Klo_ = Klo[nt]; Khi_ = Khi[nt]
                            blo = olo[nt] - olo[b0]
                            bhi = ohi[nt] - ohi[b0]
                            ohb_t = sb.tile([P, 2 * TM * P], bf, tag="ohb")
                            nc.sync.dma_start(
                                out=ohb_t[:, 0:2 * T_ * P],
                                in_=ohb_d[:, 2 * od[nt] * P:2 * (od[nt] + T_) * P])
                            ohne = ohb_t[:, 0:T_ * P]
                            ohen = ohb_t[:, T_ * P:2 * T_ * P]

                            acc = ps.tile([P, HC + H], fp32, tag="acc",
                                          space="PSUM", bufs=2)
                            msg = sb.tile([P, TM, HC + H], bf, tag="msg")
                            # slot-aligned self-loop tile: z = xl + xr, scatter
                            # via identity (also keeps pad-slot denominators
                            # positive, so no NaNs on padding)
                            zs = sb.tile([P, HC], bf, tag="zs")
                            nc.vector.tensor_tensor(out=zs[:],
                                                    in0=xl1_all[:, nt, :],
                                                    in1=xr1_all[:, nt, :],
                                                    op=OP.add)
                            ts_s = sb.tile([P, HC], bf, tag="ts_s")
                            nc.scalar.activation(out=ts_s[:], in_=zs[:],
                                                 func=AF.Prelu, alpha=alpha)
                            tas = sb.tile([P, HC], bf, tag="tas")
                            nc.vector.tensor_tensor(out=tas[:], in0=ts_s[:],
                                                    in1=attBK[:, 0:HC],
                                                    op=OP.mult)
                            msgs = sb.tile([P, HC + H], bf, tag="msgs")
                            scs = sb.tile([P, H], fp32, tag="scs")
                            nc.vector.tensor_reduce(
                                out=scs[:],
                                in_=tas[:].rearrange("p (h c) -> p h c", h=H),
                                axis=AX.X, op=OP.add)
                            nc.scalar.activation(out=msgs[:, HC:HC + H],
                                                 in_=scs[:], func=AF.Exp)
                            nc.vector.tensor_tensor(
                                out=msgs[:, 0:HC].rearrange(
                                    "p (h c) -> p h c", h=H),
                                in0=xl1_all[:, nt, :].rearrange(
                                    "p (h c) -> p h c", h=H),
                                in1=msgs[:, HC:HC + H][:, :, None]
                                    .to_broadcast([P, H, CH]),
                                op=OP.mult)
                            nc.tensor.matmul(out=acc[:], lhsT=ident,
                                             rhs=msgs[:],
                                             start=True, stop=(T_ == 0))
                            for t0, K, gx, gb0 in [(0, Klo_, glo, blo),
                                                   (Klo_, Khi_, ghi, bhi)]:
                                if K == 0:
                                    continue
                                t_sb = sb.tile([P, KM, HC], bf, tag="t")
                                for jj in range(0, K, 3):
                                    je = min(jj + 3, K)
                                    zc = ps.tile([P, 3, HC], fp32, tag="z",
                                                 space="PSUM", bufs=2)
                                    for j in range(jj, je):
                                        nc.tensor.matmul(
                                            out=zc[:, j - jj, :],
                                            lhsT=ohne[:, (t0 + j) * P:(t0 + j + 1) * P],
                                            rhs=xr1_all[:, nt, :],
                                            start=True, stop=False)
                                        nc.tensor.matmul(
                                            out=zc[:, j - jj, :], lhsT=ident,
                                            rhs=gx[:, gb0 + j, :],
                                            start=False, stop=True)
                                    nc.scalar.activation(out=t_sb[:, jj:je, :],
                                                         in_=zc[:, 0:je - jj, :],
                                                         func=AF.Prelu, alpha=alpha)
                                ta = sb.tile([P, KM, HC], bf, tag="ta")
                                nc.vector.tensor_tensor(
                                    out=ta[:, 0:K, :], in0=t_sb[:, 0:K, :],
                                    in1=attBK[:, 0:K * HC].rearrange(
                                        "p (k c) -> p k c", k=K), op=OP.mult)
                                sc = sb.tile([P, KM * H], fp32, tag="sc")
                                nc.vector.tensor_reduce(
                                    out=sc[:, 0:K * H],
                                    in_=ta[:, 0:K, :].rearrange(
                                        "p k (h c) -> p (k h) c", h=H),
                                    axis=AX.X, op=OP.add)
                                nc.scalar.activation(
                                    out=msg[:, t0:t0 + K, HC:HC + H],
                                    in_=sc[:, 0:K * H], func=AF.Exp)
                                nc.vector.tensor_tensor(
                                    out=msg[:, t0:t0 + K, 0:HC].rearrange(
                                        "p k (h c) -> p k h c", h=H),
                                    in0=gx[:, gb0:gb0 + K, :].rearrange(
                                        "p k (h c) -> p k h c", h=H),
                                    in1=msg[:, t0:t0 + K, HC:HC + H]
                                        [:, :, :, None]
                                        .to_broadcast([P, K, H, CH]),
                                    op=OP.mult)
                                for j in range(K):
                                    nc.tensor.matmul(
                                        out=acc[:],
                                        lhsT=ohen[:, (t0 + j) * P:(t0 + j + 1) * P],
                                        rhs=msg[:, t0 + j, :],
                                        start=False, stop=(t0 + j == T_ - 1))

                            rec = sb.tile([P, H], fp32, tag="rec")
                            nc.vector.reciprocal(out=rec[:], in_=acc[:, HC:HC + H])
                            h1 = sb.tile([P, HC], fp32, tag="h1")
                            nc.vector.tensor_tensor(
                                out=h1[:].rearrange("p (h c) -> p h c", h=H),
                                in0=acc[:, 0:HC].rearrange("p (h c) -> p h c", h=H),
                                in1=rec[:, :, None].to_broadcast([P, H, CH]),
                                op=OP.mult)
                            if dims["add_b1"]:
                                nc.vector.tensor_tensor(out=h1[:], in0=h1[:],
                                                        in1=b1B, op=OP.add)
                            # elu(x) = relu(x) - relu(1 - exp(x))
                            eh = sb.tile([P, HC], fp32, tag="eh")
                            nc.scalar.activation(out=eh[:], in_=h1[:], func=AF.Exp)
                            rn = sb.tile([P, HC], fp32, tag="em")
                            nc.scalar.activation(out=rn[:], in_=eh[:], func=AF.Relu,
                                                 scale=-1.0, bias=1.0)
                            rh = sb.tile([P, HC], fp32, tag="rh")
                            nc.scalar.activation(out=rh[:], in_=h1[:], func=AF.Relu)
                            elu = sb.tile([P, HC], bf, tag="elu")
                            nc.vector.tensor_tensor(out=elu[:], in0=rh[:],
                                                    in1=rn[:], op=OP.subtract)

                            tail_ps = ps.tile([P, KH * P + 2 * CO], fp32,
                                              tag="tail", space="PSUM", bufs=2)
                            for k in range(KH):
                                nc.tensor.matmul(
                                    out=tail_ps[:, k * P:(k + 1) * P],
                                    lhsT=elu[:, k * P:(k + 1) * P],
                                    rhs=ident, start=True, stop=True)
                            hT_sb = sb.tile([P, KH, P], bf, tag="hTs")
                            nc.scalar.activation(
                                out=hT_sb[:],
                                in_=tail_ps[:, 0:KH * P].rearrange(
                                    "p (k q) -> p k q", k=KH),
                                func=AF.Copy)
                            x2_ps = tail_ps[:, KH * P:KH * P + 2 * CO]
                            for k in range(KH):
                                nc.tensor.matmul(out=x2_ps, lhsT=hT_sb[:, k, :],
                                                 rhs=w2_sb[:, k, :],
                                                 start=(k == 0), stop=(k == KH - 1))
                            nc.scalar.activation(out=xl2_all[:, nt, :],
                                                 in_=x2_ps[:, 0:CO],
                                                 func=AF.Copy)
                            nc.vector.tensor_copy(out=xr2_all[:, nt, :],
                                                  in_=x2_ps[:, CO:2 * CO])
                            gi = grp_of_nt[nt]
                            g0_, _ = groups[gi]
                            nc.sync.dma_start(
                                out=ag2_in[gi][(nt - g0_) * P:(nt - g0_ + 1) * P,
                                               0:CO],
                                in_=xl2_all[:, nt, :])
                        if PHASES >= 3 and b1 == groups[grp_of_nt[b0]][1]:
                            gi = grp_of_nt[b0]
                            g0_, g1_ = groups[gi]
                            ag_chunk(ag2_in[gi][:], tbl2[gi][:])

            # ============ phase C: layer-2 edges ============
            if PHASES >= 3:
                with (tc.tile_pool(name="sbC", bufs=2) as sb,
                      tc.tile_pool(name="gbC", bufs=3) as gb,
                      tc.tile_pool(name="psC", bufs=2, space="PSUM") as ps):
                    PF = 6

                    def issue_lo2(i):
                        b0, b1 = batchesC[i]
                        nlo_b = olo[b1] - olo[b0]
                        g2lo = gb.tile([P, BKMC, CO_PAD], bf, tag="g2lo",
                                       bufs=PF + 2)
                        gi_ = nc.gpsimd.dma_gather(
                            g2lo[:, 0:nlo_b, :], tbl2[0][:],
                            gidx_lo_sb[:, olo[b0] * 8:olo[b1] * 8],
                            nlo_b * P, regs[nlo_b], CO_PAD,
                            queue_num=1, single_packet=SINGLE_PACKET)
                        _add_dep_helper(gi_.ins, lib.ins, sync=False,
                                        reason="lib")
                        return g2lo

                    lo_pend = {}
                    for i in range(min(PF, len(batchesC))):
                        lo_pend[i] = issue_lo2(i)
                    for bi, (b0, b1) in enumerate(batchesC):
                        nlo_b = olo[b1] - olo[b0]
                        nhi_b = ohi[b1] - ohi[b0]
                        g2lo = lo_pend.pop(bi)
                        g2hi = gb.tile([P, BKMC, CO_PAD], bf, tag="g2hi")
                        g2i = nc.gpsimd.dma_gather(
                            g2hi[:, 0:nhi_b, :], tbl2[1][:],
                            gidx_hi_sb[:, ohi[b0] * 8:ohi[b1] * 8],
                            nhi_b * P, regs[nhi_b], CO_PAD,
                            queue_num=0, single_packet=SINGLE_PACKET)
                        _add_dep_helper(g2i.ins, lib.ins, sync=False,
                                        reason="lib")
                        if bi + PF < len(batchesC):
                            lo_pend[bi + PF] = issue_lo2(bi + PF)

                        for nt in range(b0, b1):
                            T_ = T[nt]; Klo_ = Klo[nt]; Khi_ = Khi[nt]
                            blo = olo[nt] - olo[b0]
                            bhi = ohi[nt] - ohi[b0]
                            ohb_t = sb.tile([P, 2 * TM * P], bf, tag="ohb2")
                            nc.sync.dma_start(
                                out=ohb_t[:, 0:2 * T_ * P],
                                in_=ohb_d[:, 2 * od[nt] * P:2 * (od[nt] + T_) * P])
                            ohne = ohb_t[:, 0:T_ * P]
                            ohen = ohb_t[:, T_ * P:2 * T_ * P]

                            acc2 = ps.tile([P, CO + 1], fp32, tag="acc2",
                                           space="PSUM", bufs=2)
                            msg2 = sb.tile([P, TM, CO + 1], bf, tag="msg2")
                            # self-loop tile (see phase B)
                            z2s = sb.tile([P, CO], bf, tag="z2s")
                            nc.vector.tensor_tensor(out=z2s[:],
                                                    in0=xl2_all[:, nt, :],
                                                    in1=xr2_all[:, nt, :],
                                                    op=OP.add)
                            t2s = sb.tile([P, CO], bf, tag="t2s")
                            nc.scalar.activation(out=t2s[:], in_=z2s[:],
                                                 func=AF.Prelu, alpha=alpha)
                            ta2s = sb.tile([P, CO], bf, tag="ta2s")
                            nc.vector.tensor_tensor(out=ta2s[:], in0=t2s[:],
                                                    in1=att2BK[:, 0:CO],
                                                    op=OP.mult)
                            msg2s = sb.tile([P, CO + 1], bf, tag="msg2s")
                            sc2s = sb.tile([P, 1], fp32, tag="sc2s")
                            nc.vector.tensor_reduce(out=sc2s[:], in_=ta2s[:],
                                                    axis=AX.X, op=OP.add)
                            nc.scalar.activation(out=msg2s[:, CO:CO + 1],
                                                 in_=sc2s[:], func=AF.Exp)
                            nc.vector.tensor_tensor(
                                out=msg2s[:, 0:CO], in0=xl2_all[:, nt, :],
                                in1=msg2s[:, CO:CO + 1].to_broadcast([P, CO]),
                                op=OP.mult)
                            nc.tensor.matmul(out=acc2[:], lhsT=ident,
                                             rhs=msg2s[:],
                                             start=True, stop=(T_ == 0))
                            for t0, K, gx, gb0 in [(0, Klo_, g2lo, blo),
                                                   (Klo_, Khi_, g2hi, bhi)]:
                                if K == 0:
                                    continue
                                t2 = sb.tile([P, KM, CO], bf, tag="t2")
                                zc = ps.tile([P, KM, CO], fp32, tag="z2",
                                             space="PSUM", bufs=2)
                                for j in range(K):
                                    nc.tensor.matmul(
                                        out=zc[:, j, :],
                                        lhsT=ohne[:, (t0 + j) * P:(t0 + j + 1) * P],
                                        rhs=xr2_all[:, nt, :],
                                        start=True, stop=True)
                                # gathered-row inject on DVE (tensor is the
                                # scarcer engine in this phase)
                                s2 = sb.tile([P, KM, CO], bf, tag="s2")
                                nc.vector.tensor_tensor(
                                    out=s2[:, 0:K, :], in0=zc[:, 0:K, :],
                                    in1=gx[:, gb0:gb0 + K, 0:CO],
                                    op=OP.add)
                                nc.scalar.activation(out=t2[:, 0:K, :],
                                                     in_=s2[:, 0:K, :],
                                                     func=AF.Prelu, alpha=alpha)
                                ta2 = sb.tile([P, KM, CO], bf, tag="ta2")
                                nc.vector.tensor_tensor(
                                    out=ta2[:, 0:K, :], in0=t2[:, 0:K, :],
                                    in1=att2BK[:, 0:K * CO].rearrange(
                                        "p (k c) -> p k c", k=K), op=OP.mult)
                                sc2 = sb.tile([P, KM], fp32, tag="sc2")
                                nc.vector.tensor_reduce(
                                    out=sc2[:, 0:K], in_=ta2[:, 0:K, :],
                                    axis=AX.X, op=OP.add)
                                nc.scalar.activation(
                                    out=msg2[:, t0:t0 + K, CO:CO + 1],
                                    in_=sc2[:, 0:K], func=AF.Exp)
                                nc.vector.tensor_tensor(
                                    out=msg2[:, t0:t0 + K, 0:CO],
                                    in0=gx[:, gb0:gb0 + K, 0:CO],
                                    in1=msg2[:, t0:t0 + K, CO:CO + 1]
                                        .to_broadcast([P, K, CO]),
                                    op=OP.mult)
                                for j in range(K):
                                    nc.tensor.matmul(
                                        out=acc2[:],
                                        lhsT=ohen[:, (t0 + j) * P:(t0 + j + 1) * P],
                                        rhs=msg2[:, t0 + j, :],
                                        start=False, stop=(t0 + j == T_ - 1))

                            rec2 = sb.tile([P, 1], fp32, tag="rec2")
                            nc.vector.reciprocal(out=rec2[:], in_=acc2[:, CO:CO + 1])
                            h2 = sb.tile([P, CO], fp32, tag="h2")
                            nc.scalar.activation(out=h2[:], in_=acc2[:, 0:CO],
                                                 func=AF.Copy,
                                                 scale=rec2[:, 0:1])
                            if dims["add_b2"]:
                                nc.vector.tensor_tensor(out=h2[:], in0=h2[:],
                                                        in1=b2B, op=OP.add)
                            nc.sync.dma_start(
                                out=h2_out.rearrange("(a p) d -> p a d", p=P)
                                    [:, nt, :],
                                in_=h2[:])
                            # h2 is O(1): skip max-subtraction in log_softmax
                            esc = sb.tile([P, CO], fp32, tag="esc")
                            ssum = sb.tile([P, 1], fp32, tag="ssum")
                            nc.scalar.activation(out=esc[:], in_=h2[:], func=AF.Exp,
                                                 accum_out=ssum[:, 0:1])
                            lns = sb.tile([P, 1], fp32, tag="lns")
                            nc.scalar.activation(out=lns[:], in_=ssum[:], func=AF.Ln)
                            ls = sb.tile([P, CO], fp32, tag="ls")
                            nc.vector.tensor_tensor(
                                out=ls[:], in0=h2[:],
                                in1=lns[:, 0:1].to_broadcast([P, CO]),
                                op=OP.subtract)
                            nc.sync.dma_start(
                                out=ls_out.rearrange("(a p) d -> p a d", p=P)
                                    [:, nt, :],
                                in_=ls[:])



    if post_passes:
        _br.generate_event_semaphores(nc)
        _br.codegen_inst_isa_subclasses(nc)
    return nc


# --------------------------------------------------------------------------
# entry point
# --------------------------------------------------------------------------

def kernel(x, edge_index, W1l, W1r, att1, b1, W2l, W2r, att2, b2):
    x = np.asarray(x, np.float32)
    edge_index = np.asarray(edge_index)
    W1l = np.asarray(W1l, np.float32); W1r = np.asarray(W1r, np.float32)
    att1 = np.asarray(att1, np.float32); b1 = np.asarray(b1, np.float32)
    W2l = np.asarray(W2l, np.float32); W2r = np.asarray(W2r, np.float32)
    att2 = np.asarray(att2, np.float32); b2 = np.asarray(b2, np.float32)

    N, DIN = x.shape
    E = edge_index.shape[1]
    H, CH = att1.shape
    HC = W1l.shape[1]
    CO = W2l.shape[1]

    key = (N, E, DIN, H, CH, HC, CO,
           int(np.abs(b1).max() > 0), int(np.abs(b2).max() > 0),
           hash(edge_index.tobytes()))
    if key in _plan_cache:
        pp, nc, dims = _plan_cache[key]
    else:
        pp = _preprocess(N, E, edge_index)
        dims = dict(DIN=DIN, HC=HC, H=H, CH=CH, CO=CO,
                    NPC=pp["NPC"], NT=pp["NT"], NTG=pp["NTG"],
                    NTG2=pp["NTG2"], TBL_G=pp["TBL_G"],
                    Klo=pp["Klo"], Khi=pp["Khi"], T=pp["T"], KM=pp["KM"],
                    od=pp["od"], olo=pp["olo"], ohi=pp["ohi"],
                    OD=pp["OD"], OLO=pp["OLO"], OHI=pp["OHI"],
                    groups=pp["groups"],
                    add_b1=bool(np.abs(b1).max() > 0),
                    add_b2=bool(np.abs(b2).max() > 0))
        nc = _build_program(dims)
        _plan_cache[key] = (pp, nc, dims)

    NPC = pp["NPC"]; NT = pp["NT"]; KM = pp["KM"]
    KD = DIN // P
    bfdt = ml_dtypes.bfloat16

    # consts blob: ident | attBK | b1B | att2BK | b2B
    ident = np.eye(P, dtype=np.float32)
    attBK = np.broadcast_to(
        np.tile(att1.reshape(HC), KM)[None, :], (P, KM * HC))
    b1B = np.broadcast_to(b1.reshape(1, HC), (P, HC))
    att2BK = np.broadcast_to(
        np.tile(att2.reshape(CO), KM)[None, :], (P, KM * CO))
    b2B = np.broadcast_to(b2.reshape(1, CO), (P, CO))
    consts = np.concatenate([ident, attBK, b1B, att2BK, b2B],
                            axis=1).astype(bfdt)
    constf = np.full((P, 1), NEG_SLOPE, np.float32)
    w1cat = np.concatenate([W1l, W1r], axis=1).astype(bfdt)
    w2cat = np.concatenate([W2l, W2r], axis=1).astype(bfdt)

    in_maps = []
    for c in range(NC):
        xkc = np.zeros((NPC, DIN), np.float32)
        sel = pp["node_order"][c]
        real = sel >= 0
        xkc[real] = x[sel[real]]
        # [p, nt, k, q] = xkc[nt*P+q, k*P+p]
        xkT = np.ascontiguousarray(
            xkc.reshape(NT, P, KD, P).transpose(3, 0, 2, 1)
        ).reshape(P, NT * KD * P).astype(bfdt)
        in_maps.append(dict(
            xkT=xkT, w1=w1cat, w2=w2cat, consts=consts, constf=constf,
            gidx_lo=np.ascontiguousarray(pp["gidx_lo"][c]),
            gidx_hi=np.ascontiguousarray(pp["gidx_hi"][c]),
            ohb=np.ascontiguousarray(pp["ohb"][c]),
        ))

    from concourse.bass_utils import run_bass_kernel_spmd
    res = run_bass_kernel_spmd(nc, in_maps, core_ids=list(range(NC)))

    h = np.empty((N, CO), np.float32)
    ls = np.empty((N, CO), np.float32)
    r_core = pp["core_of"]
    r_loc = pp["local_of"]
    for c in range(NC):
        m = r_core == c
        h[m] = res.results[c]["h2o"][r_loc[m]]
        ls[m] = res.results[c]["lso"][r_loc[m]]
    return h, ls



# revision 75
# speedup vs baseline: 1.0220x; 1.0063x over previous
"""2-layer GATv2 (PyG GATv2Conv semantics) on 8 Trainium2 NeuronCores.

Strategy (v3):
  - Nodes sharded across 8 cores; per-core greedy 2D packing balances each
    destination tile's lo/hi in-edge counts (lo/hi = source node group, one
    AllGather chunk / int16-indexable table half each).
  - x is shipped pre-transposed; layer-1 projections are 6 accumulating
    matmuls per node tile with a combined [W1l|W1r] moving operand.
  - xl tables AllGathered HBM->HBM per group, overlapped with compute.
  - Per destination tile, incoming-edge source rows are fetched with
    dma_gather (int16 idx; two table halves; GB node tiles per call).
    SWDGE descriptor generation is serial on the Q7 at ~9ns/row and is the
    hard floor of this design -- hence self-loops are NOT gathered: each
    node tile has a dedicated slot-aligned self tile computed from resident
    xl/xr SBUF copies (DVE add + Prelu + identity-scatter), which also keeps
    pad-slot denominators positive (no NaNs).  Both one-hots (oh_ne dst
    -major, oh_en edge-major) are precomputed on host, streamed from HBM.
  - Per half (K edge tiles): K z-matmul pairs (one-hot xr broadcast +
    identity gx inject) into a shared PSUM chunk, ONE batched Prelu per <=3
    tiles, batched att mult / reduce / exp / msg mult over the half, then K
    scatter matmuls accumulate numerator+denominator in PSUM.
  - elu(x) = relu(x) - relu(1 - exp(x)): 3 ACT ops + 1 DVE op.
  - Softmax skips max-subtraction (scores O(1)); log_softmax likewise.
  - Layer 2 (heads=1, 16 ch) repeats the edge structure on a 256B-row
    table; epilogue division/log-softmax moved to ACT (scale/bias) where
    possible.

kernel(**inputs) takes FULL inputs, returns FULL outputs.
"""

import os
import sys

if "/opt/trn_rl_repo" not in sys.path:
    sys.path.insert(0, "/opt/trn_rl_repo")

import numpy as np
import ml_dtypes

NC = 8          # cores
P = 128         # partitions
NEG_SLOPE = 0.2
NGRP = 2        # AllGather chunks

_plan_cache = {}


# --------------------------------------------------------------------------
# host-side graph preprocessing
# --------------------------------------------------------------------------

def _snake(order, nbins):
    n = len(order)
    ids = np.arange(n)
    round_ = ids // nbins
    pos = ids % nbins
    b = np.where(round_ % 2 == 0, pos, nbins - 1 - pos)
    out = np.empty(n, np.int64)
    out[:] = b
    return out


def _preprocess(N, E, edge_index):
    # Two rank-groups (= AllGather chunks = table halves), 25 tile-ranks each.
    # (An unequal 40/60 split was tried to start the gather stream earlier; it
    # is 270us WORSE: the bigger second AllGather delays the hi-half tables
    # that phase B consumes tile-by-tile. Equal halves are optimal.)
    NTG = ((N + 2 * NC - 1) // (2 * NC) + P - 1) // P      # tiles per group
    NT = 2 * NTG
    NPC = NT * P
    NTG0 = NT // 2
    NTG1 = NT - NTG0
    NTGS = (NTG0, NTG1)
    NTGM = max(NTGS)
    TBL_G = NC * NTGM * P                                  # rows per half-table
    assert TBL_G < 32768, "table half must fit int16 row indices"

    # self-loops are handled by a dedicated slot-aligned "self tile" per node
    # tile in the kernel (no gather needed) -- only real edges go in lists.
    src = edge_index[0].astype(np.int64)
    dst = edge_index[1].astype(np.int64)
    deg = np.bincount(dst, minlength=N)

    # --- group assignment (fixes each edge's table half), then cores within
    # each group balanced by OWN-group in-degree (the heavy, self-loop half)
    order = np.argsort(-deg, kind="stable")
    grp_of = np.empty(N, np.int64)
    grp_of[order] = _snake(order, 2)
    e_own = grp_of[src] == grp_of[dst]
    deg_own = np.bincount(dst[e_own], minlength=N)
    deg_oth = deg - deg_own
    core_of = np.empty(N, np.int64)
    for g in range(2):
        nodes_g = np.where(grp_of == g)[0]
        og = nodes_g[np.argsort(-deg_own[nodes_g], kind="stable")]
        core_of[og] = _snake(og, NC)

    lo_src = grp_of[src] == 0              # which table half each edge reads

    # --- per (core, group) greedy packing into NTG tiles: keep the heavy
    # (own-group) sum under 5*P and the light sum under 4*P per tile
    slot_of = np.empty(N, np.int64)
    tile_of = np.empty(N, np.int64)        # tile index within the group
    nheav = np.zeros((NC, 2, NTGM), np.int64)
    nlite = np.zeros((NC, 2, NTGM), np.int64)
    cnt_ct = np.zeros((NC, 2, NTGM), np.int64)
    for c in range(NC):
        for g in range(2):
            nodes = np.where((core_of == c) & (grp_of == g))[0]
            nodes = nodes[np.argsort(
                -(deg_own[nodes] * 64 + deg_oth[nodes]), kind="stable")]
            hv = np.zeros(NTGS[g], np.int64)
            lt = np.zeros(NTGS[g], np.int64)
            cnt = np.zeros(NTGS[g], np.int64)
            for v in nodes:
                cost = (hv + deg_own[v]).astype(np.float64) \
                    + 0.02 * (lt + deg_oth[v]) + 1e-4 * cnt \
                    + 1e6 * np.maximum(lt + deg_oth[v] - 4 * P, 0)
                cost[cnt >= P] = 1e18
                t = int(np.argmin(cost))
                tile_of[v] = t
                slot_of[v] = cnt[t]
                cnt[t] += 1
                hv[t] += deg_own[v]
                lt[t] += deg_oth[v]
            nheav[c, g, :NTGS[g]] = hv
            nlite[c, g, :NTGS[g]] = lt
            cnt_ct[c, g, :NTGS[g]] = cnt

    # --- per (core, group) rank permutation to align heavy tiles
    kh_ = (nheav + P - 1) // P
    kl_ = (nlite + P - 1) // P
    perm = np.zeros((NC, 2, NTGM), np.int64)
    for c in range(NC):
        for g in range(2):
            ng = NTGS[g]
            key = (kh_[c, g, :ng] + kl_[c, g, :ng]) + 1e-3 * kh_[c, g, :ng] \
                + 1e-9 * (nheav[c, g, :ng] + nlite[c, g, :ng])
            perm[c, g, :ng] = np.argsort(-key, kind="stable")

    # rank r in [0, NTG) -> group 0, [NTG, NT) -> group 1
    # group 0 tiles: heavy half = lo;  group 1 tiles: heavy half = hi
    Klo = []
    Khi = []
    for r in range(NT):
        g, rr = (0, r) if r < NTG0 else (1, r - NTG0)
        kh = int(max(kh_[c, g, perm[c, g, rr]] for c in range(NC)))
        kl = int(max(kl_[c, g, perm[c, g, rr]] for c in range(NC)))
        if g == 0:
            Klo.append(kh); Khi.append(kl)
        else:
            Klo.append(kl); Khi.append(kh)
    T = [Klo[r] + Khi[r] for r in range(NT)]
    KM = max(max(Klo), max(Khi))

    rank_of = np.zeros((NC, 2, NTGM), np.int64)
    for c in range(NC):
        for g in range(2):
            rank_of[c, g, perm[c, g, :NTGS[g]]] = np.arange(NTGS[g])
    rank_glob = rank_of[core_of, grp_of, tile_of] + grp_of * NTG0
    local_of = rank_glob * P + slot_of
    # table row within the node's half-table: [core][rank-in-group][slot]
    NTG2 = (NTG + 1) // 2
    ntg_of = np.where(grp_of == 0, NTG0, NTG1)
    row_half = core_of * ntg_of * P + rank_of[core_of, grp_of, tile_of] * P \
        + slot_of

    # --- per (core, group, tile) edge lists split by half
    e_core = core_of[dst]
    e_grp = grp_of[dst]
    e_tile = tile_of[dst]
    e_slot = slot_of[dst]
    lists_lo = {}
    lists_hi = {}
    for c in range(NC):
        for g in range(2):
            m_cg = (e_core == c) & (e_grp == g)
            for tl in range(NTGS[g]):
                m = m_cg & (e_tile == tl)
                ml = m & lo_src
                mh = m & ~lo_src
                lists_lo[(c, g, tl)] = (row_half[src[ml]], e_slot[ml])
                lists_hi[(c, g, tl)] = (row_half[src[mh]], e_slot[mh])

    # offsets
    od = np.concatenate([[0], np.cumsum(T)]).astype(np.int64)
    olo = np.concatenate([[0], np.cumsum(Klo)]).astype(np.int64)
    ohi = np.concatenate([[0], np.cumsum(Khi)]).astype(np.int64)
    OD = int(od[-1]); OLO = int(olo[-1]); OHI = int(ohi[-1])

    def pack_idx(flat):
        n = len(flat)
        s = (n + 15) // 16
        arr = np.zeros(s * 16, np.int16)
        arr[:n] = flat
        block = arr.reshape(s, 16).T
        return np.tile(block, (8, 1))

    gidx_lo = np.zeros((NC, P, OLO * 8), np.int16)
    gidx_hi = np.zeros((NC, P, OHI * 8), np.int16)
    drel = np.full((NC, P, OD), -1.0, np.float32)
    for c in range(NC):
        for r in range(NT):
            g, rr = (0, r) if r < NTG0 else (1, r - NTG0)
            tl = perm[c, g, rr]
            for K, off8, dcol0, lst, gax in [
                    (Klo[r], olo[r], od[r], lists_lo[(c, g, tl)], gidx_lo),
                    (Khi[r], ohi[r], od[r] + Klo[r], lists_hi[(c, g, tl)],
                     gidx_hi)]:
                rows, slots = lst
                n = len(rows)
                assert n <= K * P
                flat = np.zeros(K * P, np.int64)
                flat[:n] = rows
                gax[c, :, off8 * 8:(off8 + K) * 8] = pack_idx(flat)
                dr = np.full(K * P, -1.0, np.float32)
                dr[:n] = slots
                drel[c, :, dcol0:dcol0 + K] = dr.reshape(K, P).T

    node_order = np.full((NC, NPC), -1, np.int64)
    for c in range(NC):
        nodes = np.where(core_of == c)[0]
        node_order[c, local_of[nodes]] = nodes

    # per tile col: [ohne (dst-major [d, e]) | ohen (edge-major [e, d])]
    ar = np.arange(P, dtype=np.float32)
    ohne = (ar[None, :, None, None]
            == drel.transpose(0, 2, 1)[:, None, :, :])
    ohen = (drel[:, :, :, None] == ar[None, None, None, :])
    ohb = np.empty((NC, P, 2 * OD * P), ml_dtypes.bfloat16)
    for r in range(NT):
        o0, o1 = int(od[r]), int(od[r + 1])
        t_ = o1 - o0
        ohb[:, :, 2 * o0 * P:(2 * o0 + t_) * P] = \
            ohne[:, :, o0:o1, :].reshape(NC, P, t_ * P)
        ohb[:, :, (2 * o0 + t_) * P:2 * o1 * P] = \
            ohen[:, :, o0:o1, :].reshape(NC, P, t_ * P)

    groups = [(0, NTG0), (NTG0, NT)]

    return dict(NPC=NPC, NT=NT, NTG=NTG, NTG2=NTG2, TBL_G=TBL_G,
                Klo=Klo, Khi=Khi, T=T, KM=KM,
                od=od.tolist(), olo=olo.tolist(), ohi=ohi.tolist(),
                OD=OD, OLO=OLO, OHI=OHI, groups=groups,
                gidx_lo=gidx_lo, gidx_hi=gidx_hi, ohb=ohb,
                node_order=node_order, core_of=core_of, local_of=local_of)


# --------------------------------------------------------------------------
# bass program
# --------------------------------------------------------------------------

def _build_program(dims, post_passes=True):
    PHASES = int(os.environ.get("GAT_PHASES", "3"))
    SHARED = os.environ.get("GAT_SHARED", "1") == "1"
    GB = int(os.environ.get("GAT_GB", "2"))        # gather batch (node tiles)
    SINGLE_PACKET = os.environ.get("GAT_SP", "0") == "1"
    INJ_ACT = os.environ.get("GAT_INJ", "mm") == "act"
    import concourse.bass as bass
    import concourse.mybir as mybir
    import concourse.tile as tile
    from concourse import library_config
    from concourse.bass import _add_dep_helper
    import bass_rust as _br

    fp32 = mybir.dt.float32
    bf = mybir.dt.bfloat16
    i16 = mybir.dt.int16
    AX = mybir.AxisListType
    OP = mybir.AluOpType
    AF = mybir.ActivationFunctionType

    DIN = dims["DIN"]; HC = dims["HC"]; H = dims["H"]; CH = dims["CH"]
    CO = dims["CO"]
    NPC = dims["NPC"]; NT = dims["NT"]; NTG = dims["NTG"]
    NTG2 = dims["NTG2"]
    TBL_G = dims["TBL_G"]
    Klo = dims["Klo"]; Khi = dims["Khi"]; T = dims["T"]; KM = dims["KM"]
    od = dims["od"]; olo = dims["olo"]; ohi = dims["ohi"]
    OD = dims["OD"]; OLO = dims["OLO"]; OHI = dims["OHI"]
    groups = dims["groups"]
    KD = DIN // P
    KH = HC // P
    CO_PAD = 128
    TM = max(T)
    addr_space = "Shared" if SHARED else "Local"

    # gather batches: consecutive ranks within each AG group, <= GB tiles
    batches = []        # (nt0, nt1)
    for g0, g1 in groups:
        nt = g0
        while nt < g1:
            batches.append((nt, min(nt + GB, g1)))
            nt = batches[-1][1]
    BKM = max(max(olo[b1] - olo[b0], ohi[b1] - ohi[b0]) for b0, b1 in batches)
    # phase C uses coarser gather batches (paired within each group) to halve
    # the serial per-gather fixed cost on the Q7
    batchesC = []
    for g0_, g1_ in groups:
        gbs = [b for b in batches if g0_ <= b[0] < g1_]
        i = 0
        while i < len(gbs):
            if i + 1 < len(gbs):
                batchesC.append((gbs[i][0], gbs[i + 1][1]))
                i += 2
            else:
                batchesC.append(gbs[i])
                i += 1
    BKMC = max(max(olo[b1] - olo[b0], ohi[b1] - ohi[b0]) for b0, b1 in batchesC)

    nc = bass.Bass(num_devices=NC, num_swdge_queues=4,
                   dynamic_dma_scratch_size=int(os.environ.get("GAT_DDS", "16384")))

    xkT_d = nc.dram_tensor("xkT", [P, NT * KD * P], bf, kind="ExternalInput")
    w1_d = nc.dram_tensor("w1", [DIN, 2 * HC], bf, kind="ExternalInput")
    w2_d = nc.dram_tensor("w2", [HC, 2 * CO], bf, kind="ExternalInput")
    CCOLS = KM * HC + HC + KM * CO + CO + P
    consts = nc.dram_tensor("consts", [P, CCOLS], bf, kind="ExternalInput")
    constf = nc.dram_tensor("constf", [P, 1], fp32, kind="ExternalInput")
    gidx_lo_d = nc.dram_tensor("gidx_lo", [P, OLO * 8], i16, kind="ExternalInput")
    gidx_hi_d = nc.dram_tensor("gidx_hi", [P, OHI * 8], i16, kind="ExternalInput")
    ohb_d = nc.dram_tensor("ohb", [P, 2 * OD * P], bf, kind="ExternalInput")
    h2_out = nc.dram_tensor("h2o", [NPC, CO], fp32, kind="ExternalOutput")
    ls_out = nc.dram_tensor("lso", [NPC, CO], fp32, kind="ExternalOutput")

    with tile.TileContext(nc) as tc:
        with (
            tc.tile_pool(name="dram", bufs=1, space="DRAM") as dram,
            tc.tile_pool(name="cst", bufs=1) as cst,
        ):
            lib = nc.gpsimd.load_library(library_config.mlp)
            regs = {}
            for b0, b1 in batches + batchesC:
                for n in (olo[b1] - olo[b0], ohi[b1] - ohi[b0]):
                    if n not in regs:
                        regs[n] = nc.gpsimd.to_reg(n * P)

            ctile = cst.tile([P, CCOLS], bf)
            nc.sync.dma_start(out=ctile[:], in_=consts[:])
            cftile = cst.tile([P, 1], fp32)
            nc.sync.dma_start(out=cftile[:], in_=constf[:])
            o = 0
            ident = ctile[:, o:o + P]; o += P
            attBK = ctile[:, o:o + KM * HC]; o += KM * HC
            b1B = ctile[:, o:o + HC]; o += HC
            att2BK = ctile[:, o:o + KM * CO]; o += KM * CO
            b2B = ctile[:, o:o + CO]; o += CO
            alpha = cftile[:, 0:1]

            w1_sb = cst.tile([P, KD, 2 * HC], bf)
            nc.sync.dma_start(out=w1_sb[:], in_=w1_d.rearrange("(k p) c -> p k c", p=P))
            w2_sb = cst.tile([P, KH, 2 * CO], bf)
            nc.sync.dma_start(out=w2_sb[:], in_=w2_d.rearrange("(k p) c -> p k c", p=P))

            gidx_lo_sb = cst.tile([P, OLO * 8], i16)
            nc.sync.dma_start(out=gidx_lo_sb[:], in_=gidx_lo_d[:])
            gidx_hi_sb = cst.tile([P, OHI * 8], i16)
            nc.sync.dma_start(out=gidx_hi_sb[:], in_=gidx_hi_d[:])

            xr1_all = cst.tile([P, NT, HC], bf)
            xr2_all = cst.tile([P, NT, CO], bf)
            xl1_all = cst.tile([P, NT, HC], bf)
            xl2_all = cst.tile([P, NT, CO], bf)
            nc.vector.memset(xr2_all[:], 0.0)

            tbl1 = {}
            tbl2 = {}
            ag1_in = {}
            ag2_in = {}
            for gi, (g0, g1) in enumerate(groups):
                rows = (g1 - g0) * P
                ag1_in[gi] = dram.tile([rows, HC], bf, name=f"ag1i_{gi}")
                ag2_in[gi] = dram.tile([rows, CO_PAD], bf, name=f"ag2i_{gi}")
                tbl1[gi] = dram.tile([NC * rows, HC], bf,
                                     addr_space=addr_space, name=f"tbl1_{gi}")
                tbl2[gi] = dram.tile([NC * rows, CO_PAD], bf,
                                     addr_space=addr_space, name=f"tbl2_{gi}")

            def ag_chunk(src, dst):
                nc.gpsimd.collective_compute(
                    "AllGather", mybir.AluOpType.bypass,
                    replica_groups=[list(range(NC))],
                    ins=[src.opt()],
                    outs=[dst.opt()],
                )



            # ============ phase A: layer-1 projections ============
            with (tc.tile_pool(name="sbA", bufs=3) as sb,
                  tc.tile_pool(name="psA", bufs=2, space="PSUM") as ps):
                XB = 4
                for gi, (g0, g1) in enumerate(groups):
                    for nb in range(g0, g1, XB):
                        ne = min(nb + XB, g1)
                        xt = sb.tile([P, XB, KD, P], bf, tag="xt")
                        nc.sync.dma_start(
                            out=xt[:, 0:ne - nb, :, :],
                            in_=xkT_d[:, nb * KD * P:ne * KD * P])
                        for nt in range(nb, ne):
                            xlr_ps = ps.tile([P, 2 * HC], fp32, tag="mm",
                                             space="PSUM")
                            for k in range(KD):
                                nc.tensor.matmul(out=xlr_ps[:],
                                                 lhsT=xt[:, nt - nb, k, :],
                                                 rhs=w1_sb[:, k, :],
                                                 start=(k == 0),
                                                 stop=(k == KD - 1))
                            nc.scalar.activation(out=xl1_all[:, nt, :],
                                                 in_=xlr_ps[:, 0:HC],
                                                 func=AF.Copy)
                            nc.vector.tensor_copy(out=xr1_all[:, nt, :],
                                                  in_=xlr_ps[:, HC:2 * HC])
                            nc.sync.dma_start(
                                out=ag1_in[gi][(nt - g0) * P:
                                               (nt - g0 + 1) * P, :],
                                in_=xl1_all[:, nt, :])
                    if PHASES >= 2:
                        ag_chunk(ag1_in[gi][:], tbl1[gi][:])

            # ============ phase B: layer-1 edges ============
            if PHASES >= 2:
                grp_of_nt = {}
                for gi, (g0, g1) in enumerate(groups):
                    for nt in range(g0, g1):
                        grp_of_nt[nt] = gi
                with (tc.tile_pool(name="sbB", bufs=2) as sb,
                      tc.tile_pool(name="gbB", bufs=3) as gb,
                      tc.tile_pool(name="psB", bufs=2, space="PSUM") as ps):
                    PF = 6

                    def issue_lo1(i):
                        b0, b1 = batches[i]
                        nlo_b = olo[b1] - olo[b0]
                        glo = gb.tile([P, BKM, HC], bf, tag="glo",
                                      bufs=PF + 2)
                        gi_ = nc.gpsimd.dma_gather(
                            glo[:, 0:nlo_b, :], tbl1[0][:],
                            gidx_lo_sb[:, olo[b0] * 8:olo[b1] * 8],
                            nlo_b * P, regs[nlo_b], HC,
                            queue_num=1, single_packet=SINGLE_PACKET)
                        _add_dep_helper(gi_.ins, lib.ins, sync=False,
                                        reason="lib")
                        return glo

                    lo_pend = {}
                    for i in range(min(PF, len(batches))):
                        lo_pend[i] = issue_lo1(i)
                    for bi, (b0, b1) in enumerate(batches):
                        nlo_b = olo[b1] - olo[b0]
                        nhi_b = ohi[b1] - ohi[b0]
                        glo = lo_pend.pop(bi)
                        ghi = gb.tile([P, BKM, HC], bf, tag="ghi")
                        g2i = nc.gpsimd.dma_gather(
                            ghi[:, 0:nhi_b, :], tbl1[1][:],
                            gidx_hi_sb[:, ohi[b0] * 8:ohi[b1] * 8],
                            nhi_b * P, regs[nhi_b], HC,
                            queue_num=0, single_packet=SINGLE_PACKET)
                        _add_dep_helper(g2i.ins, lib.ins, sync=False,
                                        reason="lib")
                        if bi + PF < len(batches):
                            lo_pend[bi + PF] = issue_lo1(bi + PF)

                        for nt in range(b0, b1):
                            T_ = T[nt]; Klo_ = Klo[nt]; Khi_ = Khi[nt]
                            blo = olo[nt] - olo[b0]
                            bhi = ohi[nt] - ohi[b0]
                            ohb_t = sb.tile([P, 2 * TM * P], bf, tag="ohb")
                            nc.sync.dma_start(
                                out=ohb_t[:, 0:2 * T_ * P],
                                in_=ohb_d[:, 2 * od[nt] * P:2 * (od[nt] + T_) * P])
                            ohne = ohb_t[:, 0:T_ * P]
                            ohen = ohb_t[:, T_ * P:2 * T_ * P]

                            acc = ps.tile([P, HC + H], fp32, tag="acc",
                                          space="PSUM", bufs=2)
                            msg = sb.tile([P, TM, HC + H], bf, tag="msg")
                            # slot-aligned self-loop tile: z = xl + xr, scatter
                            # via identity (also keeps pad-slot denominators
                            # positive, so no NaNs on padding)
                            zs = sb.tile([P, HC], bf, tag="zs")
                            nc.vector.tensor_tensor(out=zs[:],
                                                    in0=xl1_all[:, nt, :],
                                                    in1=xr1_all[:, nt, :],
                                                    op=OP.add)
                            ts_s = sb.tile([P, HC], bf, tag="ts_s")
                            nc.scalar.activation(out=ts_s[:], in_=zs[:],
                                                 func=AF.Prelu, alpha=alpha)
                            tas = sb.tile([P, HC], bf, tag="tas")
                            nc.vector.tensor_tensor(out=tas[:], in0=ts_s[:],
                                                    in1=attBK[:, 0:HC],
                                                    op=OP.mult)
                            msgs = sb.tile([P, HC + H], bf, tag="msgs")
                            scs = sb.tile([P, H], fp32, tag="scs")
                            nc.vector.tensor_reduce(
                                out=scs[:],
                                in_=tas[:].rearrange("p (h c) -> p h c", h=H),
                                axis=AX.X, op=OP.add)
                            nc.scalar.activation(out=msgs[:, HC:HC + H],
                                                 in_=scs[:], func=AF.Exp)
                            nc.vector.tensor_tensor(
                                out=msgs[:, 0:HC].rearrange(
                                    "p (h c) -> p h c", h=H),
                                in0=xl1_all[:, nt, :].rearrange(
                                    "p (h c) -> p h c", h=H),
                                in1=msgs[:, HC:HC + H][:, :, None]
                                    .to_broadcast([P, H, CH]),
                                op=OP.mult)
                            nc.tensor.matmul(out=acc[:], lhsT=ident,
                                             rhs=msgs[:],
                                             start=True, stop=(T_ == 0))
                            for t0, K, gx, gb0 in [(0, Klo_, glo, blo),
                                                   (Klo_, Khi_, ghi, bhi)]:
                                if K == 0:
                                    continue
                                t_sb = sb.tile([P, KM, HC], bf, tag="t")
                                for jj in range(0, K, 3):
                                    je = min(jj + 3, K)
                                    zc = ps.tile([P, 3, HC], fp32, tag="z",
                                                 space="PSUM", bufs=2)
                                    for j in range(jj, je):
                                        nc.tensor.matmul(
                                            out=zc[:, j - jj, :],
                                            lhsT=ohne[:, (t0 + j) * P:(t0 + j + 1) * P],
                                            rhs=xr1_all[:, nt, :],
                                            start=True, stop=False)
                                        nc.tensor.matmul(
                                            out=zc[:, j - jj, :], lhsT=ident,
                                            rhs=gx[:, gb0 + j, :],
                                            start=False, stop=True)
                                    nc.scalar.activation(out=t_sb[:, jj:je, :],
                                                         in_=zc[:, 0:je - jj, :],
                                                         func=AF.Prelu, alpha=alpha)
                                ta = sb.tile([P, KM, HC], bf, tag="ta")
                                nc.vector.tensor_tensor(
                                    out=ta[:, 0:K, :], in0=t_sb[:, 0:K, :],
                                    in1=attBK[:, 0:K * HC].rearrange(
                                        "p (k c) -> p k c", k=K), op=OP.mult)
                                sc = sb.tile([P, KM * H], fp32, tag="sc")
                                nc.vector.tensor_reduce(
                                    out=sc[:, 0:K * H],
                                    in_=ta[:, 0:K, :].rearrange(
                                        "p k (h c) -> p (k h) c", h=H),
                                    axis=AX.X, op=OP.add)
                                nc.scalar.activation(
                                    out=msg[:, t0:t0 + K, HC:HC + H],
                                    in_=sc[:, 0:K * H], func=AF.Exp)
                                nc.vector.tensor_tensor(
                                    out=msg[:, t0:t0 + K, 0:HC].rearrange(
                                        "p k (h c) -> p k h c", h=H),
                                    in0=gx[:, gb0:gb0 + K, :].rearrange(
                                        "p k (h c) -> p k h c", h=H),
                                    in1=msg[:, t0:t0 + K, HC:HC + H]
                                        [:, :, :, None]
                                        .to_broadcast([P, K, H, CH]),
                                    op=OP.mult)
                                for j in range(K):
                                    nc.tensor.matmul(
                                        out=acc[:],
                                        lhsT=ohen[:, (t0 + j) * P:(t0 + j + 1) * P],
                                        rhs=msg[:, t0 + j, :],
                                        start=False, stop=(t0 + j == T_ - 1))

                            rec = sb.tile([P, H], fp32, tag="rec")
                            nc.vector.reciprocal(out=rec[:], in_=acc[:, HC:HC + H])
                            h1 = sb.tile([P, HC], fp32, tag="h1")
                            nc.vector.tensor_tensor(
                                out=h1[:].rearrange("p (h c) -> p h c", h=H),
                                in0=acc[:, 0:HC].rearrange("p (h c) -> p h c", h=H),
                                in1=rec[:, :, None].to_broadcast([P, H, CH]),
                                op=OP.mult)
                            if dims["add_b1"]:
                                nc.vector.tensor_tensor(out=h1[:], in0=h1[:],
                                                        in1=b1B, op=OP.add)
                            # elu(x) = relu(x) - relu(1 - exp(x))
                            eh = sb.tile([P, HC], fp32, tag="eh")
                            nc.scalar.activation(out=eh[:], in_=h1[:], func=AF.Exp)
                            rn = sb.tile([P, HC], fp32, tag="em")
                            nc.scalar.activation(out=rn[:], in_=eh[:], func=AF.Relu,
                                                 scale=-1.0, bias=1.0)
                            rh = sb.tile([P, HC], fp32, tag="rh")
                            nc.scalar.activation(out=rh[:], in_=h1[:], func=AF.Relu)
                            elu = sb.tile([P, HC], bf, tag="elu")
                            nc.vector.tensor_tensor(out=elu[:], in0=rh[:],
                                                    in1=rn[:], op=OP.subtract)

                            tail_ps = ps.tile([P, KH * P + 2 * CO], fp32,
                                              tag="tail", space="PSUM", bufs=2)
                            for k in range(KH):
                                nc.tensor.matmul(
                                    out=tail_ps[:, k * P:(k + 1) * P],
                                    lhsT=elu[:, k * P:(k + 1) * P],
                                    rhs=ident, start=True, stop=True)
                            hT_sb = sb.tile([P, KH, P], bf, tag="hTs")
                            nc.scalar.activation(
                                out=hT_sb[:],
                                in_=tail_ps[:, 0:KH * P].rearrange(
                                    "p (k q) -> p k q", k=KH),
                                func=AF.Copy)
                            x2_ps = tail_ps[:, KH * P:KH * P + 2 * CO]
                            for k in range(KH):
                                nc.tensor.matmul(out=x2_ps, lhsT=hT_sb[:, k, :],
                                                 rhs=w2_sb[:, k, :],
                                                 start=(k == 0), stop=(k == KH - 1))
                            nc.scalar.activation(out=xl2_all[:, nt, :],
                                                 in_=x2_ps[:, 0:CO],
                                                 func=AF.Copy)
                            nc.vector.tensor_copy(out=xr2_all[:, nt, :],
                                                  in_=x2_ps[:, CO:2 * CO])
                            gi = grp_of_nt[nt]
                            g0_, _ = groups[gi]
                            nc.sync.dma_start(
                                out=ag2_in[gi][(nt - g0_) * P:(nt - g0_ + 1) * P,
                                               0:CO],
                                in_=xl2_all[:, nt, :])
                        if PHASES >= 3 and b1 == groups[grp_of_nt[b0]][1]:
                            gi = grp_of_nt[b0]
                            g0_, g1_ = groups[gi]
                            ag_chunk(ag2_in[gi][:], tbl2[gi][:])

            # ============ phase C: layer-2 edges ============
            if PHASES >= 3:
                with (tc.tile_pool(name="sbC", bufs=2) as sb,
                      tc.tile_pool(name="gbC", bufs=3) as gb,
                      tc.tile_pool(name="psC", bufs=2, space="PSUM") as ps):
                    PF = 6

                    def issue_lo2(i):
                        b0, b1 = batchesC[i]
                        nlo_b = olo[b1] - olo[b0]
                        g2lo = gb.tile([P, BKMC, CO_PAD], bf, tag="g2lo",
                                       bufs=PF + 2)
                        gi_ = nc.gpsimd.dma_gather(
                            g2lo[:, 0:nlo_b, :], tbl2[0][:],
                            gidx_lo_sb[:, olo[b0] * 8:olo[b1] * 8],
                            nlo_b * P, regs[nlo_b], CO_PAD,
                            queue_num=1, single_packet=SINGLE_PACKET)
                        _add_dep_helper(gi_.ins, lib.ins, sync=False,
                                        reason="lib")
                        return g2lo

                    lo_pend = {}
                    for i in range(min(PF, len(batchesC))):
                        lo_pend[i] = issue_lo2(i)
                    for bi, (b0, b1) in enumerate(batchesC):
                        nlo_b = olo[b1] - olo[b0]
                        nhi_b = ohi[b1] - ohi[b0]
                        g2lo = lo_pend.pop(bi)
                        g2hi = gb.tile([P, BKMC, CO_PAD], bf, tag="g2hi")
                        g2i = nc.gpsimd.dma_gather(
                            g2hi[:, 0:nhi_b, :], tbl2[1][:],
                            gidx_hi_sb[:, ohi[b0] * 8:ohi[b1] * 8],
                            nhi_b * P, regs[nhi_b], CO_PAD,
                            queue_num=0, single_packet=SINGLE_PACKET)
                        _add_dep_helper(g2i.ins, lib.ins, sync=False,
                                        reason="lib")
                        if bi + PF < len(batchesC):
                            lo_pend[bi + PF] = issue_lo2(bi + PF)

                        for nt in range(b0, b1):
                            T_ = T[nt]; Klo_ = Klo[nt]; Khi_ = Khi[nt]
                            blo = olo[nt] - olo[b0]
                            bhi = ohi[nt] - ohi[b0]
                            ohb_t = sb.tile([P, 2 * TM * P], bf, tag="ohb2")
                            nc.sync.dma_start(
                                out=ohb_t[:, 0:2 * T_ * P],
                                in_=ohb_d[:, 2 * od[nt] * P:2 * (od[nt] + T_) * P])
                            ohne = ohb_t[:, 0:T_ * P]
                            ohen = ohb_t[:, T_ * P:2 * T_ * P]

                            acc2 = ps.tile([P, CO + 1], fp32, tag="acc2",
                                           space="PSUM", bufs=2)
                            msg2 = sb.tile([P, TM, CO + 1], bf, tag="msg2")
                            # self-loop tile (see phase B)
                            z2s = sb.tile([P, CO], bf, tag="z2s")
                            nc.vector.tensor_tensor(out=z2s[:],
                                                    in0=xl2_all[:, nt, :],
                                                    in1=xr2_all[:, nt, :],
                                                    op=OP.add)
                            t2s = sb.tile([P, CO], bf, tag="t2s")
                            nc.scalar.activation(out=t2s[:], in_=z2s[:],
                                                 func=AF.Prelu, alpha=alpha)
                            ta2s = sb.tile([P, CO], bf, tag="ta2s")
                            nc.vector.tensor_tensor(out=ta2s[:], in0=t2s[:],
                                                    in1=att2BK[:, 0:CO],
                                                    op=OP.mult)
                            msg2s = sb.tile([P, CO + 1], bf, tag="msg2s")
                            sc2s = sb.tile([P, 1], fp32, tag="sc2s")
                            nc.vector.tensor_reduce(out=sc2s[:], in_=ta2s[:],
                                                    axis=AX.X, op=OP.add)
                            nc.scalar.activation(out=msg2s[:, CO:CO + 1],
                                                 in_=sc2s[:], func=AF.Exp)
                            nc.vector.tensor_tensor(
                                out=msg2s[:, 0:CO], in0=xl2_all[:, nt, :],
                                in1=msg2s[:, CO:CO + 1].to_broadcast([P, CO]),
                                op=OP.mult)
                            nc.tensor.matmul(out=acc2[:], lhsT=ident,
                                             rhs=msg2s[:],
                                             start=True, stop=(T_ == 0))
                            for t0, K, gx, gb0 in [(0, Klo_, g2lo, blo),
                                                   (Klo_, Khi_, g2hi, bhi)]:
                                if K == 0:
                                    continue
                                t2 = sb.tile([P, KM, CO], bf, tag="t2")
                                zc = ps.tile([P, KM, CO], fp32, tag="z2",
                                             space="PSUM", bufs=2)
                                for j in range(K):
                                    nc.tensor.matmul(
                                        out=zc[:, j, :],
                                        lhsT=ohne[:, (t0 + j) * P:(t0 + j + 1) * P],
                                        rhs=xr2_all[:, nt, :],
                                        start=True, stop=False)
                                    nc.tensor.matmul(
                                        out=zc[:, j, :], lhsT=ident,
                                        rhs=gx[:, gb0 + j, 0:CO],
                                        start=False, stop=True)
                                nc.scalar.activation(out=t2[:, 0:K, :],
                                                     in_=zc[:, 0:K, :],
                                                     func=AF.Prelu, alpha=alpha)
                                ta2 = sb.tile([P, KM, CO], bf, tag="ta2")
                                nc.vector.tensor_tensor(
                                    out=ta2[:, 0:K, :], in0=t2[:, 0:K, :],
                                    in1=att2BK[:, 0:K * CO].rearrange(
                                        "p (k c) -> p k c", k=K), op=OP.mult)
                                sc2 = sb.tile([P, KM], fp32, tag="sc2")
                                nc.vector.tensor_reduce(
                                    out=sc2[:, 0:K], in_=ta2[:, 0:K, :],
                                    axis=AX.X, op=OP.add)
                                nc.scalar.activation(
                                    out=msg2[:, t0:t0 + K, CO:CO + 1],
                                    in_=sc2[:, 0:K], func=AF.Exp)
                                nc.vector.tensor_tensor(
                                    out=msg2[:, t0:t0 + K, 0:CO],
                                    in0=gx[:, gb0:gb0 + K, 0:CO],
                                    in1=msg2[:, t0:t0 + K, CO:CO + 1]
                                        .to_broadcast([P, K, CO]),
                                    op=OP.mult)
                                for j in range(K):
                                    nc.tensor.matmul(
                                        out=acc2[:],
                                        lhsT=ohen[:, (t0 + j) * P:(t0 + j + 1) * P],
                                        rhs=msg2[:, t0 + j, :],
                                        start=False, stop=(t0 + j == T_ - 1))

                            rec2 = sb.tile([P, 1], fp32, tag="rec2")
                            nc.vector.reciprocal(out=rec2[:], in_=acc2[:, CO:CO + 1])
                            h2 = sb.tile([P, CO], fp32, tag="h2")
                            nc.scalar.activation(out=h2[:], in_=acc2[:, 0:CO],
                                                 func=AF.Copy,
                                                 scale=rec2[:, 0:1])
                            if dims["add_b2"]:
                                nc.vector.tensor_tensor(out=h2[:], in0=h2[:],
                                                        in1=b2B, op=OP.add)
                            nc.sync.dma_start(
                                out=h2_out.rearrange("(a p) d -> p a d", p=P)
                                    [:, nt, :],
                                in_=h2[:])
                            # h2 is O(1): skip max-subtraction in log_softmax
                            esc = sb.tile([P, CO], fp32, tag="esc")
                            ssum = sb.tile([P, 1], fp32, tag="ssum")
                            nc.scalar.activation(out=esc[:], in_=h2[:], func=AF.Exp,
                                                 accum_out=ssum[:, 0:1])
                            lns = sb.tile([P, 1], fp32, tag="lns")
                            nc.scalar.activation(out=lns[:], in_=ssum[:], func=AF.Ln)
                            ls = sb.tile([P, CO], fp32, tag="ls")
                            nc.vector.tensor_tensor(
                                out=ls[:], in0=h2[:],
                                in1=lns[:, 0:1].to_broadcast([P, CO]),
                                op=OP.subtract)
                            nc.sync.dma_start(
                                out=ls_out.rearrange("(a p) d -> p a d", p=P)
                                    [:, nt, :],
                                in_=ls[:])



    if post_passes:
        _br.generate_event_semaphores(nc)
        _br.codegen_inst_isa_subclasses(nc)
    return nc


# --------------------------------------------------------------------------
# entry point
# --------------------------------------------------------------------------

def kernel(x, edge_index, W1l, W1r, att1, b1, W2l, W2r, att2, b2):
    x = np.asarray(x, np.float32)
    edge_index = np.asarray(edge_index)
    W1l = np.asarray(W1l, np.float32); W1r = np.asarray(W1r, np.float32)
    att1 = np.asarray(att1, np.float32); b1 = np.asarray(b1, np.float32)
    W2l = np.asarray(W2l, np.float32); W2r = np.asarray(W2r, np.float32)
    att2 = np.asarray(att2, np.float32); b2 = np.asarray(b2, np.float32)

    N, DIN = x.shape
    E = edge_index.shape[1]
    H, CH = att1.shape
    HC = W1l.shape[1]
    CO = W2l.shape[1]

    key = (N, E, DIN, H, CH, HC, CO,
           int(np.abs(b1).max() > 0), int(np.abs(b2).max() > 0),
           hash(edge_index.tobytes()))
    if key in _plan_cache:
        pp, nc, dims = _plan_cache[key]
    else:
        pp = _preprocess(N, E, edge_index)
        dims = dict(DIN=DIN, HC=HC, H=H, CH=CH, CO=CO,
                    NPC=pp["NPC"], NT=pp["NT"], NTG=pp["NTG"],
                    NTG2=pp["NTG2"], TBL_G=pp["TBL_G"],
                    Klo=pp["Klo"], Khi=pp["Khi"], T=pp["T"], KM=pp["KM"],
                    od=pp["od"], olo=pp["olo"], ohi=pp["ohi"],
                    OD=pp["OD"], OLO=pp["OLO"], OHI=pp["OHI"],
                    groups=pp["groups"],
                    add_b1=bool(np.abs(b1).max() > 0),
                    add_b2=bool(np.abs(b2).max() > 0))
        nc = _build_program(dims)
        _plan_cache[key] = (pp, nc, dims)

    NPC = pp["NPC"]; NT = pp["NT"]; KM = pp["KM"]
    KD = DIN // P
    bfdt = ml_dtypes.bfloat16

    # consts blob: ident | attBK | b1B | att2BK | b2B
    ident = np.eye(P, dtype=np.float32)
    attBK = np.broadcast_to(
        np.tile(att1.reshape(HC), KM)[None, :], (P, KM * HC))
    b1B = np.broadcast_to(b1.reshape(1, HC), (P, HC))
    att2BK = np.broadcast_to(
        np.tile(att2.reshape(CO), KM)[None, :], (P, KM * CO))
    b2B = np.broadcast_to(b2.reshape(1, CO), (P, CO))
    consts = np.concatenate([ident, attBK, b1B, att2BK, b2B],
                            axis=1).astype(bfdt)
    constf = np.full((P, 1), NEG_SLOPE, np.float32)
    w1cat = np.concatenate([W1l, W1r], axis=1).astype(bfdt)
    w2cat = np.concatenate([W2l, W2r], axis=1).astype(bfdt)

    in_maps = []
    for c in range(NC):
        xkc = np.zeros((NPC, DIN), np.float32)
        sel = pp["node_order"][c]
        real = sel >= 0
        xkc[real] = x[sel[real]]
        # [p, nt, k, q] = xkc[nt*P+q, k*P+p]
        xkT = np.ascontiguousarray(
            xkc.reshape(NT, P, KD, P).transpose(3, 0, 2, 1)
        ).reshape(P, NT * KD * P).astype(bfdt)
        in_maps.append(dict(
            xkT=xkT, w1=w1cat, w2=w2cat, consts=consts, constf=constf,
            gidx_lo=np.ascontiguousarray(pp["gidx_lo"][c]),
            gidx_hi=np.ascontiguousarray(pp["gidx_hi"][c]),
            ohb=np.ascontiguousarray(pp["ohb"][c]),
        ))

    from concourse.bass_utils import run_bass_kernel_spmd
    res = run_bass_kernel_spmd(nc, in_maps, core_ids=list(range(NC)))

    h = np.empty((N, CO), np.float32)
    ls = np.empty((N, CO), np.float32)
    r_core = pp["core_of"]
    r_loc = pp["local_of"]
    for c in range(NC):
        m = r_core == c
        h[m] = res.results[c]["h2o"][r_loc[m]]
        ls[m] = res.results[c]["lso"][r_loc[m]]
    return h, ls

